# revision 10
# baseline (speedup 1.0000x reference)
"""CSILoss (contrastive + rotation CE) Trainium2 kernel.

Contract: kernel(**inputs) takes the FULL unsharded inputs
  z: [8192, 256] f32, rotation_predictions: [8192, 4] f32, labels: [8192] i64
and returns the full scalar loss (f32), computed on 8 NeuronCores.

Sharding: data-parallel over rows of z. Each core receives the full z (to
build the normalized-transposed embedding matrix znT used as the matmul RHS)
plus its own 1024-row slab (LHS source, rotation slab, label one-hots). Each
core computes its 1024x8192 cosine-similarity slab on the PE (fp8 DoubleRow),
exponentiates with fused row-sum accumulation on the scalar engine, extracts
the positive/diagonal terms from recomputed diagonal blocks, and reduces to
one scalar partial; the host sums the 8 partials.
"""

import sys

for _p in ("/opt/trn_rl_repo", "/root/.axon_site/_ro/trn_rl_repo"):
    if _p not in sys.path:
        sys.path.insert(0, _p)

import numpy as np

import concourse.bass as bass
import concourse.tile as tile
from concourse import bacc, mybir
from concourse.bass import ds, ts
from concourse.bass_utils import run_bass_kernel_spmd

B, D = 8192, 256
N_CORES = 8
SLAB = B // N_CORES  # 1024 rows per core
RB = SLAB // 128  # 8 row-blocks per core
TB = B // 128  # 64 total row-blocks
F32 = mybir.dt.float32
BF16 = mybir.dt.bfloat16
FP8 = mybir.dt.float8e4
AF = mybir.ActivationFunctionType
ALU = mybir.AluOpType
DR = mybir.MatmulPerfMode.DoubleRow

_CACHE = {}


def _build():
    nc = bacc.Bacc("TRN2", target_bir_lowering=False, debug=False)

    z = nc.declare_dram_parameter("z", [B, D], F32, isOutput=False)
    zslab = nc.declare_dram_parameter("zslab", [SLAB, D], F32, isOutput=False)
    rp = nc.declare_dram_parameter("rp", [SLAB, 4], F32, isOutput=False)
    oh = nc.declare_dram_parameter("oh", [SLAB, 4], F32, isOutput=False)
    idm = nc.declare_dram_parameter("idm", [128, 128], F32, isOutput=False)
    pm = nc.declare_dram_parameter("pm", [128, 128], F32, isOutput=False)
    partial = nc.declare_dram_parameter("partial", [1, 1], F32, isOutput=True)

    with tile.TileContext(nc) as tc:
        from contextlib import ExitStack

        with ExitStack() as stk:
            const = stk.enter_context(tc.tile_pool(name="const", bufs=1))
            small = stk.enter_context(tc.tile_pool(name="small", bufs=1))
            escp = stk.enter_context(tc.tile_pool(name="esc", bufs=2))
            zin = stk.enter_context(tc.tile_pool(name="zin", bufs=9))
            sqp = stk.enter_context(tc.tile_pool(name="sqp", bufs=4))
            drp = stk.enter_context(tc.tile_pool(name="drp", bufs=6))
            msc = stk.enter_context(tc.tile_pool(name="msc", bufs=2))
            psp = stk.enter_context(tc.tile_pool(name="psp", bufs=2, space="PSUM"))

            # ---- constants / small inputs
            idm_sb = const.tile([128, 128], F32)
            nc.sync.dma_start(out=idm_sb[:], in_=idm[:])
            pm_sb = const.tile([128, 128], F32)
            nc.sync.dma_start(out=pm_sb[:], in_=pm[:])
            rp_sb = const.tile([128, RB, 4], F32)
            nc.sync.dma_start(out=rp_sb[:], in_=rp[:, :].rearrange("(b p) f -> p b f", p=128))
            oh_sb = const.tile([128, RB, 4], F32)
            nc.sync.dma_start(out=oh_sb[:], in_=oh[:, :].rearrange("(b p) f -> p b f", p=128))
            ones = const.tile([128, 1], F32)
            nc.vector.memset(ones[:], 1.0)

            # persistent fp8 normalized-transposed embeddings: [p, h, col]
            znT8 = const.tile([128, 2, B], FP8, tag="znT8")
            zsT8 = const.tile([128, 2, SLAB], FP8, tag="zsT8")

            sumsq = small.tile([128, TB], F32)
            rnorm = small.tile([128, TB], F32)
            sumsq_s = small.tile([128, RB], F32)
            rnorm_s = small.tile([128, RB], F32)
            posv = small.tile([128, RB], F32)
            diagv = small.tile([128, RB], F32)
            acc = small.tile([128, RB, 4], F32)

            def rnorm_of(dst, src, sl):
                # dst = min(exp(-0.5*ln(src)), 1e8) == 1/max(sqrt(src), 1e-8)
                nc.scalar.activation(out=dst[:, sl], in_=src[:, sl], func=AF.Ln)
                nc.scalar.activation(out=dst[:, sl], in_=dst[:, sl], func=AF.Exp, scale=-0.5)
                nc.vector.tensor_scalar_min(out=dst[:, sl], in0=dst[:, sl], scalar1=1e8)

            def sumsq_of(dst_col, src_ap, eng):
                scr = sqp.tile([128, D], F32, tag=f"sq{eng}")
                e = nc.vector if eng == "v" else nc.gpsimd
                e.scalar_tensor_tensor(
                    out=scr[:], in0=src_ap, scalar=1.0, in1=src_ap,
                    op0=ALU.mult, op1=ALU.mult, accum_out=dst_col,
                )

            # ---- slab: load, norms, transpose, diagonal blocks
            zs_sb = zin.tile([128, RB, D], F32, tag="zs")
            nc.sync.dma_start(
                out=zs_sb[:], in_=zslab[:, :].rearrange("(b p) d -> p b d", p=128)
            )
            for b in range(RB):
                sumsq_of(sumsq_s[:, b : b + 1], zs_sb[:, b, :], "v")
            rnorm_of(rnorm_s, sumsq_s, slice(0, RB))

            ps_s = psp.tile([128, 2048], F32, tag="ps")
            for i in range(RB):
                dr_t = drp.tile([128, 128], F32, tag="dr")
                nc.vector.tensor_scalar_mul(
                    out=dr_t[:], in0=idm_sb[:], scalar1=rnorm_s[:, i : i + 1]
                )
                for h in range(2):
                    nc.tensor.matmul(
                        ps_s[:, ds(h * 1024 + i * 128, 128)],
                        lhsT=zs_sb[:, i, ds(h * 128, 128)],
                        rhs=dr_t[:],
                        start=True,
                        stop=True,
                    )
            for h in range(2):
                nc.vector.tensor_copy(zsT8[:, h, :], ps_s[:, ds(h * 1024, 1024)])

            # diagonal blocks (bitwise-identical recompute of the slab diagonal)
            ps_d = psp.tile([128, 2048], F32, tag="ps")
            for rb in range(RB):
                nc.tensor.matmul(
                    ps_d[:, ts(rb, 128)],
                    lhsT=zsT8[:, :, ts(rb, 128)],
                    rhs=zsT8[:, :, ts(rb, 128)],
                    start=True,
                    stop=True,
                    perf_mode=DR,
                )
            dcp = const.tile([128, RB, 128], F32)
            nc.vector.tensor_copy(dcp[:], ps_d[:, 0:1024].rearrange("p (i c) -> p i c", c=128))
            for rb in range(RB):
                mscr = msc.tile([128, 128], F32, tag="mscr")
                nc.vector.scalar_tensor_tensor(
                    out=mscr[:], in0=dcp[:, rb, :], scalar=1.0, in1=pm_sb[:],
                    op0=ALU.mult, op1=ALU.mult, accum_out=posv[:, rb : rb + 1],
                )
                mscr2 = msc.tile([128, 128], F32, tag="mscr")
                nc.vector.scalar_tensor_tensor(
                    out=mscr2[:], in0=dcp[:, rb, :], scalar=1.0, in1=idm_sb[:],
                    op0=ALU.mult, op1=ALU.mult, accum_out=diagv[:, rb : rb + 1],
                )

            # ---- full z: streamed load + norms
            z_sb = []
            for g in range(8):
                t8 = zin.tile([128, 8, D], F32, tag="zc")
                nc.sync.dma_start(
                    out=t8[:],
                    in_=z[g * 1024 : (g + 1) * 1024, :].rearrange(
                        "(b p) d -> p b d", p=128
                    ),
                )
                z_sb.append(t8)
            for g in range(8):
                for b in range(8):
                    t = 8 * g + b
                    sumsq_of(sumsq[:, t : t + 1], z_sb[g][:, b, :], "v")
                rnorm_of(rnorm, sumsq, slice(8 * g, 8 * g + 8))

            # ---- streamed: transpose chunk n, then logits+exp for chunk n
            for n in range(4):
                ps_t = [
                    psp.tile([128, 2048], F32, tag="ps", name=f"ps_t{n}_{h}")
                    for h in range(2)
                ]
                for i in range(16):
                    t = 16 * n + i
                    g, b = divmod(t, 8)
                    dr_t = drp.tile([128, 128], F32, tag="dr")
                    nc.vector.tensor_scalar_mul(
                        out=dr_t[:], in0=idm_sb[:], scalar1=rnorm[:, t : t + 1]
                    )
                    for h in range(2):
                        nc.tensor.matmul(
                            ps_t[h][:, ts(i, 128)],
                            lhsT=z_sb[g][:, b, ds(h * 128, 128)],
                            rhs=dr_t[:],
                            start=True,
                            stop=True,
                        )
                for h in range(2):
                    nc.vector.tensor_copy(znT8[:, h, ds(2048 * n, 2048)], ps_t[h][:])

                for rb in range(RB):
                    ps = psp.tile([128, 2048], F32, tag="ps")
                    for s in range(4):
                        nc.tensor.matmul(
                            ps[:, ts(s, 512)],
                            lhsT=zsT8[:, :, ts(rb, 128)],
                            rhs=znT8[:, :, ds(2048 * n + 512 * s, 512)],
                            start=True,
                            stop=True,
                            perf_mode=DR,
                        )
                    e = escp.tile([128, 2048], BF16, tag="esc")
                    nc.scalar.activation(
                        out=e[:],
                        in_=ps[:],
                        func=AF.Exp,
                        scale=4.0,
                        accum_out=acc[:, rb, n : n + 1],
                    )

            # ---- finals
            S = small.tile([128, RB], F32)
            nc.vector.reduce_sum(S[:], acc[:], axis=mybir.AxisListType.X)
            ed = small.tile([128, RB], F32)
            nc.scalar.activation(out=ed[:], in_=diagv[:], func=AF.Exp, scale=4.0)
            Sm = small.tile([128, RB], F32)
            nc.vector.tensor_tensor(out=Sm[:], in0=S[:], in1=ed[:], op=ALU.subtract)
            lse = small.tile([128, RB], F32)
            nc.scalar.activation(out=lse[:], in_=Sm[:], func=AF.Ln)
            p4 = small.tile([128, RB], F32)
            nc.vector.tensor_scalar_mul(out=p4[:], in0=posv[:], scalar1=4.0)
            lc = small.tile([128, RB], F32)
            nc.vector.tensor_tensor(out=lc[:], in0=lse[:], in1=p4[:], op=ALU.subtract)

            # rotation CE
            rs = small.tile([128, RB], F32)
            rescr = small.tile([128, RB, 4], F32)
            for b in range(RB):
                nc.scalar.activation(
                    out=rescr[:, b, :],
                    in_=rp_sb[:, b, :],
                    func=AF.Exp,
                    accum_out=rs[:, b : b + 1],
                )
            rlse = small.tile([128, RB], F32)
            nc.scalar.activation(out=rlse[:], in_=rs[:], func=AF.Ln)
            picked = small.tile([128, 1], F32)
            pscr = small.tile([128, RB, 4], F32)
            nc.vector.scalar_tensor_tensor(
                out=pscr[:], in0=rp_sb[:], scalar=1.0, in1=oh_sb[:],
                op0=ALU.mult, op1=ALU.mult, accum_out=picked[:],
            )

            csum = small.tile([128, 1], F32)
            nc.vector.reduce_sum(csum[:], lc[:], axis=mybir.AxisListType.X)
            rsum = small.tile([128, 1], F32)
            nc.vector.reduce_sum(rsum[:], rlse[:], axis=mybir.AxisListType.X)
            tot = small.tile([128, 1], F32)
            nc.vector.tensor_tensor(out=tot[:], in0=csum[:], in1=rsum[:], op=ALU.add)
            nc.vector.tensor_tensor(out=tot[:], in0=tot[:], in1=picked[:], op=ALU.subtract)

            psF = psp.tile([128, 2048], F32, tag="ps")
            nc.tensor.matmul(psF[0:1, 0:1], lhsT=tot[:], rhs=ones[:], start=True, stop=True)
            outsb = small.tile([1, 1], F32)
            nc.vector.tensor_copy(outsb[:], psF[0:1, 0:1])
            nc.sync.dma_start(out=partial[:], in_=outsb[:])

    nc.compile()
    return nc


def get_nc():
    if "nc" not in _CACHE:
        _CACHE["nc"] = _build()
    return _CACHE["nc"]


def _host_inputs(z, rotation_predictions, labels):
    z = np.ascontiguousarray(np.asarray(z, dtype=np.float32))
    rp = np.ascontiguousarray(np.asarray(rotation_predictions, dtype=np.float32))
    lab = np.asarray(labels).astype(np.int64)
    oh_full = np.eye(4, dtype=np.float32)[lab % 4]  # [B, 4]

    idm = np.eye(128, dtype=np.float32)
    pidx = np.arange(128)
    pmk = np.zeros((128, 128), dtype=np.float32)
    pmk[pidx, pidx ^ 1] = 1.0

    in_maps = []
    for c in range(N_CORES):
        r0, r1 = c * SLAB, (c + 1) * SLAB
        in_maps.append(
            {
                "z": z,
                "zslab": z[r0:r1],
                "rp": rp[r0:r1],
                "oh": oh_full[r0:r1],
                "idm": idm,
                "pm": pmk,
            }
        )
    return in_maps


def kernel(z, rotation_predictions, labels):
    nc = get_nc()
    in_maps = _host_inputs(z, rotation_predictions, labels)
    res = run_bass_kernel_spmd(nc, in_maps, core_ids=list(range(N_CORES)))
    total = sum(float(res.results[c]["partial"][0, 0]) for c in range(N_CORES))
    return np.float32(total / B)


if __name__ == "__main__":
    rng = np.random.default_rng(0)
    z = rng.standard_normal((B, D), dtype=np.float32)
    rp = rng.standard_normal((B, 4), dtype=np.float32)
    lab = rng.integers(0, 4, size=(B,)).astype(np.int64)
    print("loss:", kernel(z, rp, lab))


# revision 11
# speedup vs baseline: 1.1589x; 1.1589x over previous
"""CSILoss (contrastive + rotation CE) Trainium2 kernel.

Contract: kernel(**inputs) takes the FULL unsharded inputs
  z: [8192, 256] f32, rotation_predictions: [8192, 4] f32, labels: [8192] i64
and returns the full scalar loss (f32), computed on 8 NeuronCores.

Sharding: data-parallel over rows of z. Each core receives the full z (to
build the normalized-transposed embedding matrix znT used as the matmul RHS)
plus its own 1024-row slab (LHS source, rotation slab, label one-hots). Each
core computes its 1024x8192 cosine-similarity slab on the PE (fp8 DoubleRow),
exponentiates with fused row-sum accumulation on the scalar engine, extracts
the positive/diagonal terms from bitwise-identical recomputed diagonal
blocks, and reduces to one scalar partial; the host sums the 8 partials.

Engine split: GpSimd converts z to bf16 and builds diag(rnorm) tiles; DVE
does row sum-of-squares (fused multiply+accumulate), PSUM->fp8 copies and
mask extractions; PE does bf16 transpose matmuls + fp8 logits matmuls; the
scalar engine is reserved for Exp/Ln (one activation-table set pair, loads
grouped), with fused row-sum accumulation on the big exponentials.
"""

import sys

for _p in ("/opt/trn_rl_repo", "/root/.axon_site/_ro/trn_rl_repo"):
    if _p not in sys.path:
        sys.path.insert(0, _p)

import numpy as np

import concourse.bass as bass
import concourse.tile as tile
from concourse import bacc, mybir
from concourse.bass import ds, ts
from concourse.bass_utils import run_bass_kernel_spmd

B, D = 8192, 256
N_CORES = 8
SLAB = B // N_CORES  # 1024 rows per core
RB = SLAB // 128  # 8 row-blocks per core
TB = B // 128  # 64 total row-blocks
F32 = mybir.dt.float32
BF16 = mybir.dt.bfloat16
FP8 = mybir.dt.float8e4
AF = mybir.ActivationFunctionType
ALU = mybir.AluOpType
DR = mybir.MatmulPerfMode.DoubleRow

_CACHE = {}


def _build():
    nc = bacc.Bacc("TRN2", target_bir_lowering=False, debug=False)

    z = nc.declare_dram_parameter("z", [B, D], F32, isOutput=False)
    zslab = nc.declare_dram_parameter("zslab", [SLAB, D], F32, isOutput=False)
    rp = nc.declare_dram_parameter("rp", [SLAB, 4], F32, isOutput=False)
    oh = nc.declare_dram_parameter("oh", [SLAB, 4], F32, isOutput=False)
    idm = nc.declare_dram_parameter("idm", [128, 128], F32, isOutput=False)
    idmb = nc.declare_dram_parameter("idmb", [128, 128], BF16, isOutput=False)
    pm = nc.declare_dram_parameter("pm", [128, 128], F32, isOutput=False)
    partial = nc.declare_dram_parameter("partial", [1, 1], F32, isOutput=True)

    with tile.TileContext(nc) as tc:
        from contextlib import ExitStack

        with ExitStack() as stk:
            const = stk.enter_context(tc.tile_pool(name="const", bufs=1))
            small = stk.enter_context(tc.tile_pool(name="small", bufs=1))
            escp = stk.enter_context(tc.tile_pool(name="esc", bufs=2))
            zf32 = stk.enter_context(tc.tile_pool(name="zf32", bufs=3))
            zbfp = stk.enter_context(tc.tile_pool(name="zbfp", bufs=9))
            sqp = stk.enter_context(tc.tile_pool(name="sqp", bufs=4))
            drp = stk.enter_context(tc.tile_pool(name="drp", bufs=6))
            msc = stk.enter_context(tc.tile_pool(name="msc", bufs=2))
            psp = stk.enter_context(tc.tile_pool(name="psp", bufs=2, space="PSUM"))

            # ---- constants / small inputs
            idm_sb = const.tile([128, 128], F32)
            nc.sync.dma_start(out=idm_sb[:], in_=idm[:])
            idmb_sb = const.tile([128, 128], BF16)
            nc.sync.dma_start(out=idmb_sb[:], in_=idmb[:])
            pm_sb = const.tile([128, 128], F32)
            nc.sync.dma_start(out=pm_sb[:], in_=pm[:])
            rp_sb = const.tile([128, RB, 4], F32)
            nc.sync.dma_start(out=rp_sb[:], in_=rp[:, :].rearrange("(b p) f -> p b f", p=128))
            oh_sb = const.tile([128, RB, 4], F32)
            nc.sync.dma_start(out=oh_sb[:], in_=oh[:, :].rearrange("(b p) f -> p b f", p=128))
            ones = const.tile([128, 1], F32)
            nc.vector.memset(ones[:], 1.0)

            # persistent fp8 normalized-transposed embeddings: [p, h, col]
            znT8 = const.tile([128, 2, B], FP8, tag="znT8")
            zsT8 = const.tile([128, 2, SLAB], FP8, tag="zsT8")

            sumsq = small.tile([128, TB], F32)
            rnorm = small.tile([128, TB], F32)
            sumsq_s = small.tile([128, RB], F32)
            rnorm_s = small.tile([128, RB], F32)
            posv = small.tile([128, RB], F32)
            diagv = small.tile([128, RB], F32)
            acc = small.tile([128, RB, 4], F32)

            def sumsq_of(dst_col, src_ap):
                scr = sqp.tile([128, D], BF16, tag="sqscr")
                nc.vector.scalar_tensor_tensor(
                    out=scr[:], in0=src_ap, scalar=1.0, in1=src_ap,
                    op0=ALU.mult, op1=ALU.mult, accum_out=dst_col,
                )

            # ---- slab + first two z chunks: load, convert, sumsq
            zs_sb = zf32.tile([128, RB, D], F32, tag="zf")
            nc.sync.dma_start(
                out=zs_sb[:], in_=zslab[:, :].rearrange("(b p) d -> p b d", p=128)
            )
            zsbf = zbfp.tile([128, RB, D], BF16, tag="zsbf")
            nc.gpsimd.tensor_copy(zsbf[:], zs_sb[:])
            for b in range(RB):
                sumsq_of(sumsq_s[:, b : b + 1], zsbf[:, b, :])

            zbf = [None] * 8

            def load_chunk(g):
                zf = zf32.tile([128, 8, D], F32, tag="zf")
                nc.sync.dma_start(
                    out=zf[:],
                    in_=z[g * 1024 : (g + 1) * 1024, :].rearrange(
                        "(b p) d -> p b d", p=128
                    ),
                )
                zbf[g] = zbfp.tile([128, 8, D], BF16, tag="zbf", name=f"zbf{g}")
                nc.gpsimd.tensor_copy(zbf[g][:], zf[:])
                for b in range(8):
                    t = 8 * g + b
                    sumsq_of(sumsq[:, t : t + 1], zbf[g][:, b, :])

            load_chunk(0)
            load_chunk(1)

            # rnorm batch 1 (slab + blocks 0..15): Ln ops grouped, Exp ops grouped
            # rnorm = min(exp(-0.5*ln(s)), 1e8) == 1/max(sqrt(s), 1e-8)
            nc.scalar.activation(out=rnorm_s[:], in_=sumsq_s[:], func=AF.Ln)
            nc.scalar.activation(out=rnorm[:, 0:16], in_=sumsq[:, 0:16], func=AF.Ln)
            nc.scalar.activation(out=rnorm_s[:], in_=rnorm_s[:], func=AF.Exp, scale=-0.5)
            nc.scalar.activation(out=rnorm[:, 0:16], in_=rnorm[:, 0:16], func=AF.Exp, scale=-0.5)
            nc.vector.tensor_scalar_min(out=rnorm_s[:], in0=rnorm_s[:], scalar1=1e8)
            nc.vector.tensor_scalar_min(out=rnorm[:, 0:16], in0=rnorm[:, 0:16], scalar1=1e8)

            # rotation exps early (table already on Exp; ACT otherwise idle here)
            rs = small.tile([128, RB], F32)
            rescr = small.tile([128, RB, 4], F32)
            for b in range(RB):
                nc.scalar.activation(
                    out=rescr[:, b, :],
                    in_=rp_sb[:, b, :],
                    func=AF.Exp,
                    accum_out=rs[:, b : b + 1],
                )

            # ---- slab transpose (bf16) + diagonal blocks
            ps_s = psp.tile([128, 2048], F32, tag="ps")
            for i in range(RB):
                dr_t = drp.tile([128, 128], BF16, tag="dr")
                nc.gpsimd.tensor_scalar_mul(
                    out=dr_t[:], in0=idmb_sb[:], scalar1=rnorm_s[:, i : i + 1]
                )
                for h in range(2):
                    nc.tensor.matmul(
                        ps_s[:, ds(h * 1024 + i * 128, 128)],
                        lhsT=zsbf[:, i, ds(h * 128, 128)],
                        rhs=dr_t[:],
                        start=True,
                        stop=True,
                    )
            for h in range(2):
                nc.vector.tensor_copy(zsT8[:, h, :], ps_s[:, ds(h * 1024, 1024)])

            ps_d = psp.tile([128, 2048], F32, tag="ps")
            for rb in range(RB):
                nc.tensor.matmul(
                    ps_d[:, ts(rb, 128)],
                    lhsT=zsT8[:, :, ts(rb, 128)],
                    rhs=zsT8[:, :, ts(rb, 128)],
                    start=True,
                    stop=True,
                    perf_mode=DR,
                )
            dcp = const.tile([128, RB, 128], F32)
            nc.vector.tensor_copy(dcp[:], ps_d[:, 0:1024].rearrange("p (i c) -> p i c", c=128))
            for rb in range(RB):
                mscr = msc.tile([128, 128], F32, tag="mscr")
                nc.vector.scalar_tensor_tensor(
                    out=mscr[:], in0=dcp[:, rb, :], scalar=1.0, in1=pm_sb[:],
                    op0=ALU.mult, op1=ALU.mult, accum_out=posv[:, rb : rb + 1],
                )
                mscr2 = msc.tile([128, 128], F32, tag="mscr")
                nc.vector.scalar_tensor_tensor(
                    out=mscr2[:], in0=dcp[:, rb, :], scalar=1.0, in1=idm_sb[:],
                    op0=ALU.mult, op1=ALU.mult, accum_out=diagv[:, rb : rb + 1],
                )
            # exp of diagonal logits (Exp table is loaded; do it early)
            ed = small.tile([128, RB], F32)
            nc.scalar.activation(out=ed[:], in_=diagv[:], func=AF.Exp, scale=4.0)

            # ---- remaining z chunks + rnorm batch 2
            for g in range(2, 8):
                load_chunk(g)
            nc.scalar.activation(out=rnorm[:, 16:64], in_=sumsq[:, 16:64], func=AF.Ln)
            nc.scalar.activation(out=rnorm[:, 16:64], in_=rnorm[:, 16:64], func=AF.Exp, scale=-0.5)
            nc.vector.tensor_scalar_min(out=rnorm[:, 16:64], in0=rnorm[:, 16:64], scalar1=1e8)

            # ---- streamed: transpose chunk n (bf16), then logits+exp chunk n
            for n in range(4):
                ps_t = [
                    psp.tile([128, 2048], F32, tag="ps", name=f"ps_t{n}_{h}")
                    for h in range(2)
                ]
                for i in range(16):
                    t = 16 * n + i
                    g, b = divmod(t, 8)
                    dr_t = drp.tile([128, 128], BF16, tag="dr")
                    nc.gpsimd.tensor_scalar_mul(
                        out=dr_t[:], in0=idmb_sb[:], scalar1=rnorm[:, t : t + 1]
                    )
                    for h in range(2):
                        nc.tensor.matmul(
                            ps_t[h][:, ts(i, 128)],
                            lhsT=zbf[g][:, b, ds(h * 128, 128)],
                            rhs=dr_t[:],
                            start=True,
                            stop=True,
                        )
                for h in range(2):
                    nc.vector.tensor_copy(znT8[:, h, ds(2048 * n, 2048)], ps_t[h][:])

                for rb in range(RB):
                    ps = psp.tile([128, 2048], F32, tag="ps")
                    for s in range(4):
                        nc.tensor.matmul(
                            ps[:, ts(s, 512)],
                            lhsT=zsT8[:, :, ts(rb, 128)],
                            rhs=znT8[:, :, ds(2048 * n + 512 * s, 512)],
                            start=True,
                            stop=True,
                            perf_mode=DR,
                        )
                    e = escp.tile([128, 2048], BF16, tag="esc")
                    nc.scalar.activation(
                        out=e[:],
                        in_=ps[:],
                        func=AF.Exp,
                        scale=4.0,
                        accum_out=acc[:, rb, n : n + 1],
                    )

            # ---- finals (Ln ops grouped at the end)
            S = small.tile([128, RB], F32)
            nc.vector.reduce_sum(S[:], acc[:], axis=mybir.AxisListType.X)
            Sm = small.tile([128, RB], F32)
            nc.vector.tensor_tensor(out=Sm[:], in0=S[:], in1=ed[:], op=ALU.subtract)
            lse = small.tile([128, RB], F32)
            nc.scalar.activation(out=lse[:], in_=Sm[:], func=AF.Ln)
            rlse = small.tile([128, RB], F32)
            nc.scalar.activation(out=rlse[:], in_=rs[:], func=AF.Ln)

            p4 = small.tile([128, RB], F32)
            nc.vector.tensor_scalar_mul(out=p4[:], in0=posv[:], scalar1=4.0)
            lc = small.tile([128, RB], F32)
            nc.vector.tensor_tensor(out=lc[:], in0=lse[:], in1=p4[:], op=ALU.subtract)
            picked = small.tile([128, 1], F32)
            pscr = small.tile([128, RB, 4], F32)
            nc.vector.scalar_tensor_tensor(
                out=pscr[:], in0=rp_sb[:], scalar=1.0, in1=oh_sb[:],
                op0=ALU.mult, op1=ALU.mult, accum_out=picked[:],
            )
            csum = small.tile([128, 1], F32)
            nc.vector.reduce_sum(csum[:], lc[:], axis=mybir.AxisListType.X)
            rsum = small.tile([128, 1], F32)
            nc.vector.reduce_sum(rsum[:], rlse[:], axis=mybir.AxisListType.X)
            tot = small.tile([128, 1], F32)
            nc.vector.tensor_tensor(out=tot[:], in0=csum[:], in1=rsum[:], op=ALU.add)
            nc.vector.tensor_tensor(out=tot[:], in0=tot[:], in1=picked[:], op=ALU.subtract)

            psF = psp.tile([128, 2048], F32, tag="ps")
            nc.tensor.matmul(psF[0:1, 0:1], lhsT=tot[:], rhs=ones[:], start=True, stop=True)
            outsb = small.tile([1, 1], F32)
            nc.vector.tensor_copy(outsb[:], psF[0:1, 0:1])
            nc.sync.dma_start(out=partial[:], in_=outsb[:])

    nc.compile()
    return nc


def get_nc():
    if "nc" not in _CACHE:
        _CACHE["nc"] = _build()
    return _CACHE["nc"]


def _host_inputs(z, rotation_predictions, labels):
    import ml_dtypes

    z = np.ascontiguousarray(np.asarray(z, dtype=np.float32))
    rp = np.ascontiguousarray(np.asarray(rotation_predictions, dtype=np.float32))
    lab = np.asarray(labels).astype(np.int64)
    oh_full = np.eye(4, dtype=np.float32)[lab % 4]  # [B, 4]

    idm = np.eye(128, dtype=np.float32)
    idmb = np.eye(128, dtype=ml_dtypes.bfloat16)
    pidx = np.arange(128)
    pmk = np.zeros((128, 128), dtype=np.float32)
    pmk[pidx, pidx ^ 1] = 1.0

    in_maps = []
    for c in range(N_CORES):
        r0, r1 = c * SLAB, (c + 1) * SLAB
        in_maps.append(
            {
                "z": z,
                "zslab": z[r0:r1],
                "rp": rp[r0:r1],
                "oh": oh_full[r0:r1],
                "idm": idm,
                "idmb": idmb,
                "pm": pmk,
            }
        )
    return in_maps


def kernel(z, rotation_predictions, labels):
    nc = get_nc()
    in_maps = _host_inputs(z, rotation_predictions, labels)
    res = run_bass_kernel_spmd(nc, in_maps, core_ids=list(range(N_CORES)))
    total = sum(float(res.results[c]["partial"][0, 0]) for c in range(N_CORES))
    return np.float32(total / B)


if __name__ == "__main__":
    rng = np.random.default_rng(0)
    z = rng.standard_normal((B, D), dtype=np.float32)
    rp = rng.standard_normal((B, 4), dtype=np.float32)
    lab = rng.integers(0, 4, size=(B,)).astype(np.int64)
    print("loss:", kernel(z, rp, lab))


# revision 13
# speedup vs baseline: 1.1613x; 1.0021x over previous
"""CSILoss (contrastive + rotation CE) Trainium2 kernel.

Contract: kernel(**inputs) takes the FULL unsharded inputs
  z: [8192, 256] f32, rotation_predictions: [8192, 4] f32, labels: [8192] i64
and returns the full scalar loss (f32), computed on 8 NeuronCores.

Sharding: data-parallel over rows of z. Each core receives the full z (to
build the normalized-transposed embedding matrix znT used as the matmul RHS)
plus its own 1024-row slab (LHS source, rotation slab, label one-hots). Each
core computes its 1024x8192 cosine-similarity slab on the PE (fp8 DoubleRow),
exponentiates with fused row-sum accumulation on the scalar engine, extracts
the positive/diagonal terms from bitwise-identical recomputed diagonal
blocks, and reduces to one scalar partial; the host sums the 8 partials.

Engine split: GpSimd converts z to bf16 and builds diag(rnorm) tiles; DVE
does row sum-of-squares (fused multiply+accumulate), PSUM->fp8 copies and
mask extractions; PE does bf16 transpose matmuls + fp8 logits matmuls; the
scalar engine is reserved for Exp/Ln (one activation-table set pair, loads
grouped), with fused row-sum accumulation on the big exponentials.
"""

import sys

for _p in ("/opt/trn_rl_repo", "/root/.axon_site/_ro/trn_rl_repo"):
    if _p not in sys.path:
        sys.path.insert(0, _p)

import numpy as np

import concourse.bass as bass
import concourse.tile as tile
from concourse import bacc, mybir
from concourse.bass import ds, ts
from concourse.bass_utils import run_bass_kernel_spmd

B, D = 8192, 256
N_CORES = 8
SLAB = B // N_CORES  # 1024 rows per core
RB = SLAB // 128  # 8 row-blocks per core
TB = B // 128  # 64 total row-blocks
F32 = mybir.dt.float32
BF16 = mybir.dt.bfloat16
FP8 = mybir.dt.float8e4
AF = mybir.ActivationFunctionType
ALU = mybir.AluOpType
DR = mybir.MatmulPerfMode.DoubleRow

_CACHE = {}


def _build():
    nc = bacc.Bacc("TRN2", target_bir_lowering=False, debug=False)

    z = nc.declare_dram_parameter("z", [B, D], F32, isOutput=False)
    zslab = nc.declare_dram_parameter("zslab", [SLAB, D], F32, isOutput=False)
    rp = nc.declare_dram_parameter("rp", [SLAB, 4], F32, isOutput=False)
    oh = nc.declare_dram_parameter("oh", [SLAB, 4], F32, isOutput=False)
    idm = nc.declare_dram_parameter("idm", [128, 128], F32, isOutput=False)
    idmb = nc.declare_dram_parameter("idmb", [128, 128], BF16, isOutput=False)
    pm = nc.declare_dram_parameter("pm", [128, 128], F32, isOutput=False)
    partial = nc.declare_dram_parameter("partial", [1, 1], F32, isOutput=True)

    with tile.TileContext(nc) as tc:
        from contextlib import ExitStack

        with ExitStack() as stk:
            const = stk.enter_context(tc.tile_pool(name="const", bufs=1))
            small = stk.enter_context(tc.tile_pool(name="small", bufs=1))
            escp = stk.enter_context(tc.tile_pool(name="esc", bufs=2))
            zf32 = stk.enter_context(tc.tile_pool(name="zf32", bufs=3))
            zbfp = stk.enter_context(tc.tile_pool(name="zbfp", bufs=9))
            sqp = stk.enter_context(tc.tile_pool(name="sqp", bufs=4))
            drp = stk.enter_context(tc.tile_pool(name="drp", bufs=6))
            msc = stk.enter_context(tc.tile_pool(name="msc", bufs=2))
            psp = stk.enter_context(tc.tile_pool(name="psp", bufs=2, space="PSUM"))

            # ---- constants / small inputs
            idm_sb = const.tile([128, 128], F32)
            nc.sync.dma_start(out=idm_sb[:], in_=idm[:])
            idmb_sb = const.tile([128, 128], BF16)
            nc.sync.dma_start(out=idmb_sb[:], in_=idmb[:])
            pm_sb = const.tile([128, 128], F32)
            nc.sync.dma_start(out=pm_sb[:], in_=pm[:])
            rp_sb = const.tile([128, RB, 4], F32)
            nc.sync.dma_start(out=rp_sb[:], in_=rp[:, :].rearrange("(b p) f -> p b f", p=128))
            oh_sb = const.tile([128, RB, 4], F32)
            nc.sync.dma_start(out=oh_sb[:], in_=oh[:, :].rearrange("(b p) f -> p b f", p=128))
            ones = const.tile([128, 1], F32)
            nc.vector.memset(ones[:], 1.0)

            # persistent fp8 normalized-transposed embeddings: [p, h, col]
            znT8 = const.tile([128, 2, B], FP8, tag="znT8")
            zsT8 = const.tile([128, 2, SLAB], FP8, tag="zsT8")

            sumsq = small.tile([128, TB], F32)
            rnorm = small.tile([128, TB], F32)
            sumsq_s = small.tile([128, RB], F32)
            rnorm_s = small.tile([128, RB], F32)
            posv = small.tile([128, RB], F32)
            diagv = small.tile([128, RB], F32)
            acc = small.tile([128, RB, 4], F32)

            def sumsq_of(dst_col, src_ap):
                scr = sqp.tile([128, D], BF16, tag="sqscr")
                nc.vector.scalar_tensor_tensor(
                    out=scr[:], in0=src_ap, scalar=1.0, in1=src_ap,
                    op0=ALU.mult, op1=ALU.mult, accum_out=dst_col,
                )

            # ---- slab + first two z chunks: load, convert, sumsq
            zs_sb = zf32.tile([128, RB, D], F32, tag="zf")
            nc.sync.dma_start(
                out=zs_sb[:], in_=zslab[:, :].rearrange("(b p) d -> p b d", p=128)
            )
            zsbf = zbfp.tile([128, RB, D], BF16, tag="zsbf")
            nc.gpsimd.tensor_copy(zsbf[:], zs_sb[:])
            for b in range(RB):
                sumsq_of(sumsq_s[:, b : b + 1], zsbf[:, b, :])

            zbf = [None] * 8

            def load_chunk(g):
                zf = zf32.tile([128, 8, D], F32, tag="zf")
                nc.sync.dma_start(
                    out=zf[:],
                    in_=z[g * 1024 : (g + 1) * 1024, :].rearrange(
                        "(b p) d -> p b d", p=128
                    ),
                )
                zbf[g] = zbfp.tile([128, 8, D], BF16, tag="zbf", name=f"zbf{g}")
                nc.gpsimd.tensor_copy(zbf[g][:], zf[:])
                for b in range(8):
                    t = 8 * g + b
                    sumsq_of(sumsq[:, t : t + 1], zbf[g][:, b, :])

            load_chunk(0)
            load_chunk(1)

            # rnorm batch 1 (slab + blocks 0..15): Ln ops grouped, Exp ops grouped
            # rnorm = min(exp(-0.5*ln(s)), 1e8) == 1/max(sqrt(s), 1e-8)
            nc.scalar.activation(out=rnorm_s[:], in_=sumsq_s[:], func=AF.Ln)
            nc.scalar.activation(out=rnorm[:, 0:16], in_=sumsq[:, 0:16], func=AF.Ln)
            nc.scalar.activation(out=rnorm_s[:], in_=rnorm_s[:], func=AF.Exp, scale=-0.5)
            nc.scalar.activation(out=rnorm[:, 0:16], in_=rnorm[:, 0:16], func=AF.Exp, scale=-0.5)
            nc.vector.tensor_scalar_min(out=rnorm_s[:], in0=rnorm_s[:], scalar1=1e8)
            nc.vector.tensor_scalar_min(out=rnorm[:, 0:16], in0=rnorm[:, 0:16], scalar1=1e8)

            # rotation exps early (table already on Exp; ACT otherwise idle here)
            rs = small.tile([128, RB], F32)
            rescr = small.tile([128, RB, 4], F32)
            for b in range(RB):
                nc.scalar.activation(
                    out=rescr[:, b, :],
                    in_=rp_sb[:, b, :],
                    func=AF.Exp,
                    accum_out=rs[:, b : b + 1],
                )

            # ---- slab transpose (bf16) + diagonal blocks
            ps_s = psp.tile([128, 2048], F32, tag="ps")
            for i in range(RB):
                dr_t = drp.tile([128, 128], BF16, tag="dr")
                nc.vector.tensor_scalar_mul(
                    out=dr_t[:], in0=idmb_sb[:], scalar1=rnorm_s[:, i : i + 1]
                )
                for h in range(2):
                    nc.tensor.matmul(
                        ps_s[:, ds(h * 1024 + i * 128, 128)],
                        lhsT=zsbf[:, i, ds(h * 128, 128)],
                        rhs=dr_t[:],
                        start=True,
                        stop=True,
                    )
            for h in range(2):
                nc.vector.tensor_copy(zsT8[:, h, :], ps_s[:, ds(h * 1024, 1024)])

            ps_d = psp.tile([128, 2048], F32, tag="ps")
            for rb in range(RB):
                nc.tensor.matmul(
                    ps_d[:, ts(rb, 128)],
                    lhsT=zsT8[:, :, ts(rb, 128)],
                    rhs=zsT8[:, :, ts(rb, 128)],
                    start=True,
                    stop=True,
                    perf_mode=DR,
                )
            dcp = const.tile([128, RB, 128], F32)
            nc.vector.tensor_copy(dcp[:], ps_d[:, 0:1024].rearrange("p (i c) -> p i c", c=128))
            for rb in range(RB):
                mscr = msc.tile([128, 128], F32, tag="mscr")
                nc.vector.scalar_tensor_tensor(
                    out=mscr[:], in0=dcp[:, rb, :], scalar=1.0, in1=pm_sb[:],
                    op0=ALU.mult, op1=ALU.mult, accum_out=posv[:, rb : rb + 1],
                )
                mscr2 = msc.tile([128, 128], F32, tag="mscr")
                nc.vector.scalar_tensor_tensor(
                    out=mscr2[:], in0=dcp[:, rb, :], scalar=1.0, in1=idm_sb[:],
                    op0=ALU.mult, op1=ALU.mult, accum_out=diagv[:, rb : rb + 1],
                )
            # exp of diagonal logits (Exp table is loaded; do it early)
            ed = small.tile([128, RB], F32)
            nc.scalar.activation(out=ed[:], in_=diagv[:], func=AF.Exp, scale=4.0)

            # ---- remaining z chunks + rnorm batch 2
            for g in range(2, 8):
                load_chunk(g)
            nc.scalar.activation(out=rnorm[:, 16:64], in_=sumsq[:, 16:64], func=AF.Ln)
            nc.scalar.activation(out=rnorm[:, 16:64], in_=rnorm[:, 16:64], func=AF.Exp, scale=-0.5)
            nc.vector.tensor_scalar_min(out=rnorm[:, 16:64], in0=rnorm[:, 16:64], scalar1=1e8)

            # ---- streamed: transpose chunk n (bf16), then logits+exp chunk n
            for n in range(4):
                ps_t = [
                    psp.tile([128, 2048], F32, tag="ps", name=f"ps_t{n}_{h}")
                    for h in range(2)
                ]
                for i in range(16):
                    t = 16 * n + i
                    g, b = divmod(t, 8)
                    dr_t = drp.tile([128, 128], BF16, tag="dr")
                    nc.vector.tensor_scalar_mul(
                        out=dr_t[:], in0=idmb_sb[:], scalar1=rnorm[:, t : t + 1]
                    )
                    for h in range(2):
                        nc.tensor.matmul(
                            ps_t[h][:, ts(i, 128)],
                            lhsT=zbf[g][:, b, ds(h * 128, 128)],
                            rhs=dr_t[:],
                            start=True,
                            stop=True,
                        )
                for h in range(2):
                    nc.vector.tensor_copy(znT8[:, h, ds(2048 * n, 2048)], ps_t[h][:])

                for rb in range(RB):
                    ps = psp.tile([128, 2048], F32, tag="ps")
                    for s in range(4):
                        nc.tensor.matmul(
                            ps[:, ts(s, 512)],
                            lhsT=zsT8[:, :, ts(rb, 128)],
                            rhs=znT8[:, :, ds(2048 * n + 512 * s, 512)],
                            start=True,
                            stop=True,
                            perf_mode=DR,
                        )
                    e = escp.tile([128, 2048], BF16, tag="esc")
                    nc.scalar.activation(
                        out=e[:],
                        in_=ps[:],
                        func=AF.Exp,
                        scale=4.0,
                        accum_out=acc[:, rb, n : n + 1],
                    )

            # ---- finals (Ln ops grouped at the end)
            S = small.tile([128, RB], F32)
            nc.vector.reduce_sum(S[:], acc[:], axis=mybir.AxisListType.X)
            Sm = small.tile([128, RB], F32)
            nc.vector.tensor_tensor(out=Sm[:], in0=S[:], in1=ed[:], op=ALU.subtract)
            lse = small.tile([128, RB], F32)
            nc.scalar.activation(out=lse[:], in_=Sm[:], func=AF.Ln)
            rlse = small.tile([128, RB], F32)
            nc.scalar.activation(out=rlse[:], in_=rs[:], func=AF.Ln)

            p4 = small.tile([128, RB], F32)
            nc.vector.tensor_scalar_mul(out=p4[:], in0=posv[:], scalar1=4.0)
            lc = small.tile([128, RB], F32)
            nc.vector.tensor_tensor(out=lc[:], in0=lse[:], in1=p4[:], op=ALU.subtract)
            picked = small.tile([128, 1], F32)
            pscr = small.tile([128, RB, 4], F32)
            nc.vector.scalar_tensor_tensor(
                out=pscr[:], in0=rp_sb[:], scalar=1.0, in1=oh_sb[:],
                op0=ALU.mult, op1=ALU.mult, accum_out=picked[:],
            )
            csum = small.tile([128, 1], F32)
            nc.vector.reduce_sum(csum[:], lc[:], axis=mybir.AxisListType.X)
            rsum = small.tile([128, 1], F32)
            nc.vector.reduce_sum(rsum[:], rlse[:], axis=mybir.AxisListType.X)
            tot = small.tile([128, 1], F32)
            nc.vector.tensor_tensor(out=tot[:], in0=csum[:], in1=rsum[:], op=ALU.add)
            nc.vector.tensor_tensor(out=tot[:], in0=tot[:], in1=picked[:], op=ALU.subtract)

            psF = psp.tile([128, 2048], F32, tag="ps")
            nc.tensor.matmul(psF[0:1, 0:1], lhsT=tot[:], rhs=ones[:], start=True, stop=True)
            outsb = small.tile([1, 1], F32)
            nc.vector.tensor_copy(outsb[:], psF[0:1, 0:1])
            nc.sync.dma_start(out=partial[:], in_=outsb[:])

    nc.compile()
    return nc


def get_nc():
    if "nc" not in _CACHE:
        _CACHE["nc"] = _build()
    return _CACHE["nc"]


def _host_inputs(z, rotation_predictions, labels):
    import ml_dtypes

    z = np.ascontiguousarray(np.asarray(z, dtype=np.float32))
    rp = np.ascontiguousarray(np.asarray(rotation_predictions, dtype=np.float32))
    lab = np.asarray(labels).astype(np.int64)
    oh_full = np.eye(4, dtype=np.float32)[lab % 4]  # [B, 4]

    idm = np.eye(128, dtype=np.float32)
    idmb = np.eye(128, dtype=ml_dtypes.bfloat16)
    pidx = np.arange(128)
    pmk = np.zeros((128, 128), dtype=np.float32)
    pmk[pidx, pidx ^ 1] = 1.0

    in_maps = []
    for c in range(N_CORES):
        r0, r1 = c * SLAB, (c + 1) * SLAB
        in_maps.append(
            {
                "z": z,
                "zslab": z[r0:r1],
                "rp": rp[r0:r1],
                "oh": oh_full[r0:r1],
                "idm": idm,
                "idmb": idmb,
                "pm": pmk,
            }
        )
    return in_maps


def kernel(z, rotation_predictions, labels):
    nc = get_nc()
    in_maps = _host_inputs(z, rotation_predictions, labels)
    res = run_bass_kernel_spmd(nc, in_maps, core_ids=list(range(N_CORES)))
    total = sum(float(res.results[c]["partial"][0, 0]) for c in range(N_CORES))
    return np.float32(total / B)


if __name__ == "__main__":
    rng = np.random.default_rng(0)
    z = rng.standard_normal((B, D), dtype=np.float32)
    rp = rng.standard_normal((B, 4), dtype=np.float32)
    lab = rng.integers(0, 4, size=(B,)).astype(np.int64)
    print("loss:", kernel(z, rp, lab))


# revision 15
# speedup vs baseline: 1.4380x; 1.2383x over previous
"""CSILoss (contrastive + rotation CE) Trainium2 kernel.

Contract: kernel(**inputs) takes the FULL unsharded inputs
  z: [8192, 256] f32, rotation_predictions: [8192, 4] f32, labels: [8192] i64
and returns the full scalar loss (f32), computed on 8 NeuronCores.

Sharding: data-parallel over rows of z. Each core receives the full z (to
build the normalized-transposed embedding matrix znT used as the matmul RHS)
plus its own 1024-row slab (LHS source, rotation slab, label one-hots). Each
core computes its 1024x8192 cosine-similarity slab on the PE (fp8 DoubleRow),
exponentiates with fused row-sum accumulation on the scalar engine, extracts
the positive/diagonal terms from bitwise-identical recomputed diagonal
blocks, and reduces to one scalar partial; the host sums the 8 partials.

Engine split: GpSimd converts z to bf16; DVE computes row sums-of-squares
(fused mul+accum), applies rnorm during a bf16 rescale, copies PSUM->fp8 and
extracts masked terms; PE transposes via identity matmuls and runs the fp8
logits matmuls; the scalar engine does Exp/Ln only (table loads grouped),
with fused row-sum accumulation on the big exponentials. Next-chunk
transposes are interleaved between the exp slots to keep ACT saturated.
"""

import sys

for _p in ("/opt/trn_rl_repo", "/root/.axon_site/_ro/trn_rl_repo"):
    if _p not in sys.path:
        sys.path.insert(0, _p)

import numpy as np

import concourse.bass as bass
import concourse.tile as tile
from concourse import bacc, mybir
from concourse.bass import ds, ts
from concourse.bass_utils import run_bass_kernel_spmd

B, D = 8192, 256
N_CORES = 8
SLAB = B // N_CORES
RB = SLAB // 128
TB = B // 128
F32 = mybir.dt.float32
BF16 = mybir.dt.bfloat16
FP8 = mybir.dt.float8e4
AF = mybir.ActivationFunctionType
ALU = mybir.AluOpType
DR = mybir.MatmulPerfMode.DoubleRow

_CACHE = {}


def _build():
    nc = bacc.Bacc("TRN2", target_bir_lowering=False, debug=False)

    z = nc.declare_dram_parameter("z", [B, D], F32, isOutput=False)
    zslab = nc.declare_dram_parameter("zslab", [SLAB, D], F32, isOutput=False)
    rp = nc.declare_dram_parameter("rp", [SLAB, 4], F32, isOutput=False)
    oh = nc.declare_dram_parameter("oh", [SLAB, 4], F32, isOutput=False)
    idm = nc.declare_dram_parameter("idm", [128, 128], F32, isOutput=False)
    idmb = nc.declare_dram_parameter("idmb", [128, 128], BF16, isOutput=False)
    pm = nc.declare_dram_parameter("pm", [128, 128], F32, isOutput=False)
    partial = nc.declare_dram_parameter("partial", [1, 1], F32, isOutput=True)

    with tile.TileContext(nc) as tc:
        from contextlib import ExitStack

        with ExitStack() as stk:
            const = stk.enter_context(tc.tile_pool(name="const", bufs=1))
            small = stk.enter_context(tc.tile_pool(name="small", bufs=1))
            escp = stk.enter_context(tc.tile_pool(name="esc", bufs=2))
            zf32 = stk.enter_context(tc.tile_pool(name="zf32", bufs=4))
            zbfp = stk.enter_context(tc.tile_pool(name="zbfp", bufs=9))
            znp = stk.enter_context(tc.tile_pool(name="znp", bufs=9))
            sqp = stk.enter_context(tc.tile_pool(name="sqp", bufs=4))
            msc = stk.enter_context(tc.tile_pool(name="msc", bufs=2))
            psp = stk.enter_context(tc.tile_pool(name="psp", bufs=2, space="PSUM"))

            # ---- constants / small inputs
            idm_sb = const.tile([128, 128], F32)
            nc.sync.dma_start(out=idm_sb[:], in_=idm[:])
            idmb_sb = const.tile([128, 128], BF16)
            nc.sync.dma_start(out=idmb_sb[:], in_=idmb[:])
            pm_sb = const.tile([128, 128], F32)
            nc.sync.dma_start(out=pm_sb[:], in_=pm[:])
            rp_sb = const.tile([128, RB, 4], F32)
            nc.sync.dma_start(out=rp_sb[:], in_=rp[:, :].rearrange("(b p) f -> p b f", p=128))
            oh_sb = const.tile([128, RB, 4], F32)
            nc.sync.dma_start(out=oh_sb[:], in_=oh[:, :].rearrange("(b p) f -> p b f", p=128))
            ones = const.tile([128, 1], F32)
            nc.vector.memset(ones[:], 1.0)

            znT8 = const.tile([128, 2, B], FP8, tag="znT8")
            zsT8 = const.tile([128, 2, SLAB], FP8, tag="zsT8")

            sumsq = small.tile([128, TB], F32)
            rnorm = small.tile([128, TB], F32)
            sumsq_s = small.tile([128, RB], F32)
            rnorm_s = small.tile([128, RB], F32)
            posv = small.tile([128, RB], F32)
            diagv = small.tile([128, RB], F32)
            acc = small.tile([128, RB, 4], F32)

            def sumsq_of(dst_col, src_ap):
                scr = sqp.tile([128, D], BF16, tag="sqscr")
                nc.vector.scalar_tensor_tensor(
                    out=scr[:], in0=src_ap, scalar=1.0, in1=src_ap,
                    op0=ALU.mult, op1=ALU.mult, accum_out=dst_col,
                )

            # normalized bf16 z per chunk (natural layout), block t scaled by rnorm_t
            znbf = [None] * 8

            def load_chunk(g):
                zf = zf32.tile([128, 8, D], F32, tag="zf", name=f"zf{g}")
                nc.sync.dma_start(
                    out=zf[:],
                    in_=z[g * 1024 : (g + 1) * 1024, :].rearrange(
                        "(b p) d -> p b d", p=128
                    ),
                )
                zb = zbfp.tile([128, 8, D], BF16, tag="zbf", name=f"zbf{g}")
                nc.gpsimd.tensor_copy(zb[:], zf[:])
                for b in range(8):
                    t = 8 * g + b
                    sumsq_of(sumsq[:, t : t + 1], zb[:, b, :])
                znbf[g] = zb

            def scale_chunk(g):
                # rescale in place is not allowed across engines; write to znp tile
                zn = znp.tile([128, 8, D], BF16, tag="znbf", name=f"znbf{g}")
                for b in range(8):
                    t = 8 * g + b
                    nc.vector.tensor_scalar_mul(
                        out=zn[:, b, :], in0=znbf[g][:, b, :], scalar1=rnorm[:, t : t + 1]
                    )
                znbf[g] = zn

            # ---- slab + chunks 0..3: load, convert, sumsq
            zs_f = zf32.tile([128, RB, D], F32, tag="zf")
            nc.sync.dma_start(
                out=zs_f[:], in_=zslab[:, :].rearrange("(b p) d -> p b d", p=128)
            )
            zs_b = zbfp.tile([128, RB, D], BF16, tag="zbf")
            nc.gpsimd.tensor_copy(zs_b[:], zs_f[:])
            for b in range(RB):
                sumsq_of(sumsq_s[:, b : b + 1], zs_b[:, b, :])
            for g in range(4):
                load_chunk(g)

            # rnorm batch 1 (slab + blocks 0..31); Ln/Exp grouped for table reuse
            nc.scalar.activation(out=rnorm_s[:], in_=sumsq_s[:], func=AF.Ln)
            nc.scalar.activation(out=rnorm[:, 0:32], in_=sumsq[:, 0:32], func=AF.Ln)
            nc.scalar.activation(out=rnorm_s[:], in_=rnorm_s[:], func=AF.Exp, scale=-0.5)
            nc.scalar.activation(out=rnorm[:, 0:32], in_=rnorm[:, 0:32], func=AF.Exp, scale=-0.5)
            nc.vector.tensor_scalar_min(out=rnorm_s[:], in0=rnorm_s[:], scalar1=1e8)
            nc.vector.tensor_scalar_min(out=rnorm[:, 0:32], in0=rnorm[:, 0:32], scalar1=1e8)

            # rotation exps early (Exp table loaded; ACT otherwise idle)
            rs = small.tile([128, RB], F32)
            rescr = small.tile([128, RB, 4], F32)
            for b in range(RB):
                nc.scalar.activation(
                    out=rescr[:, b, :],
                    in_=rp_sb[:, b, :],
                    func=AF.Exp,
                    accum_out=rs[:, b : b + 1],
                )

            # ---- slab: rescale, transpose, diagonal blocks, extractions
            zn_s = znp.tile([128, RB, D], BF16, tag="znbf")
            for b in range(RB):
                nc.vector.tensor_scalar_mul(
                    out=zn_s[:, b, :], in0=zs_b[:, b, :], scalar1=rnorm_s[:, b : b + 1]
                )
            ps_s = psp.tile([128, 2048], F32, tag="ps")
            for i in range(RB):
                for h in range(2):
                    nc.tensor.matmul(
                        ps_s[:, ds(h * 1024 + i * 128, 128)],
                        lhsT=zn_s[:, i, ds(h * 128, 128)],
                        rhs=idmb_sb[:],
                        start=True,
                        stop=True,
                    )
            for h in range(2):
                nc.vector.tensor_copy(zsT8[:, h, :], ps_s[:, ds(h * 1024, 1024)])

            ps_d = psp.tile([128, 2048], F32, tag="ps")
            for rb in range(RB):
                nc.tensor.matmul(
                    ps_d[:, ts(rb, 128)],
                    lhsT=zsT8[:, :, ts(rb, 128)],
                    rhs=zsT8[:, :, ts(rb, 128)],
                    start=True,
                    stop=True,
                    perf_mode=DR,
                )
            dcp = const.tile([128, RB, 128], F32)
            nc.vector.tensor_copy(dcp[:], ps_d[:, 0:1024].rearrange("p (i c) -> p i c", c=128))
            for rb in range(RB):
                mscr = msc.tile([128, 128], F32, tag="mscr")
                nc.vector.scalar_tensor_tensor(
                    out=mscr[:], in0=dcp[:, rb, :], scalar=1.0, in1=pm_sb[:],
                    op0=ALU.mult, op1=ALU.mult, accum_out=posv[:, rb : rb + 1],
                )
                mscr2 = msc.tile([128, 128], F32, tag="mscr")
                nc.vector.scalar_tensor_tensor(
                    out=mscr2[:], in0=dcp[:, rb, :], scalar=1.0, in1=idm_sb[:],
                    op0=ALU.mult, op1=ALU.mult, accum_out=diagv[:, rb : rb + 1],
                )
            ed = small.tile([128, RB], F32)
            nc.scalar.activation(out=ed[:], in_=diagv[:], func=AF.Exp, scale=4.0)

            # ---- transpose emission helper (chunk n covers blocks 16n..16n+15)
            def emit_T_half(n, h):
                ps_t = psp.tile([128, 2048], F32, tag="ps", name=f"ps_t{n}_{h}")
                for i in range(16):
                    t = 16 * n + i
                    g, b = divmod(t, 8)
                    nc.tensor.matmul(
                        ps_t[:, ts(i, 128)],
                        lhsT=znbf[g][:, b, ds(h * 128, 128)],
                        rhs=idmb_sb[:],
                        start=True,
                        stop=True,
                    )
                nc.vector.tensor_copy(znT8[:, h, ds(2048 * n, 2048)], ps_t[:])

            scale_chunk(0)
            scale_chunk(1)
            emit_T_half(0, 0)
            emit_T_half(0, 1)

            # ---- streamed chunks: big matmuls + exp; next chunk's transposes
            # interleaved between exp slots
            for n in range(4):
                for rb in range(RB):
                    ps = psp.tile([128, 2048], F32, tag="ps")
                    for s in range(4):
                        nc.tensor.matmul(
                            ps[:, ts(s, 512)],
                            lhsT=zsT8[:, :, ts(rb, 128)],
                            rhs=znT8[:, :, ds(2048 * n + 512 * s, 512)],
                            start=True,
                            stop=True,
                            perf_mode=DR,
                        )
                    e = escp.tile([128, 2048], BF16, tag="esc")
                    nc.scalar.activation(
                        out=e[:],
                        in_=ps[:],
                        func=AF.Exp,
                        scale=4.0,
                        accum_out=acc[:, rb, n : n + 1],
                    )
                    if n < 3 and rb == 3:
                        scale_chunk(2 * (n + 1))
                        scale_chunk(2 * (n + 1) + 1)
                    if n < 3 and rb == 4:
                        emit_T_half(n + 1, 0)
                    if n < 3 and rb == 6:
                        emit_T_half(n + 1, 1)
                if n == 0:
                    for g in range(4, 8):
                        load_chunk(g)
                    nc.scalar.activation(out=rnorm[:, 32:64], in_=sumsq[:, 32:64], func=AF.Ln)
                    nc.scalar.activation(
                        out=rnorm[:, 32:64], in_=rnorm[:, 32:64], func=AF.Exp, scale=-0.5
                    )
                    nc.vector.tensor_scalar_min(
                        out=rnorm[:, 32:64], in0=rnorm[:, 32:64], scalar1=1e8
                    )

            # ---- finals (Ln ops grouped)
            S = small.tile([128, RB], F32)
            nc.vector.reduce_sum(S[:], acc[:], axis=mybir.AxisListType.X)
            Sm = small.tile([128, RB], F32)
            nc.vector.tensor_tensor(out=Sm[:], in0=S[:], in1=ed[:], op=ALU.subtract)
            lse = small.tile([128, RB], F32)
            nc.scalar.activation(out=lse[:], in_=Sm[:], func=AF.Ln)
            rlse = small.tile([128, RB], F32)
            nc.scalar.activation(out=rlse[:], in_=rs[:], func=AF.Ln)

            p4 = small.tile([128, RB], F32)
            nc.vector.tensor_scalar_mul(out=p4[:], in0=posv[:], scalar1=4.0)
            lc = small.tile([128, RB], F32)
            nc.vector.tensor_tensor(out=lc[:], in0=lse[:], in1=p4[:], op=ALU.subtract)
            picked = small.tile([128, 1], F32)
            pscr = small.tile([128, RB, 4], F32)
            nc.vector.scalar_tensor_tensor(
                out=pscr[:], in0=rp_sb[:], scalar=1.0, in1=oh_sb[:],
                op0=ALU.mult, op1=ALU.mult, accum_out=picked[:],
            )
            csum = small.tile([128, 1], F32)
            nc.vector.reduce_sum(csum[:], lc[:], axis=mybir.AxisListType.X)
            rsum = small.tile([128, 1], F32)
            nc.vector.reduce_sum(rsum[:], rlse[:], axis=mybir.AxisListType.X)
            tot = small.tile([128, 1], F32)
            nc.vector.tensor_tensor(out=tot[:], in0=csum[:], in1=rsum[:], op=ALU.add)
            nc.vector.tensor_tensor(out=tot[:], in0=tot[:], in1=picked[:], op=ALU.subtract)

            psF = psp.tile([128, 2048], F32, tag="ps")
            nc.tensor.matmul(psF[0:1, 0:1], lhsT=tot[:], rhs=ones[:], start=True, stop=True)
            outsb = small.tile([1, 1], F32)
            nc.vector.tensor_copy(outsb[:], psF[0:1, 0:1])
            nc.sync.dma_start(out=partial[:], in_=outsb[:])

    nc.compile()
    return nc


def get_nc():
    if "nc" not in _CACHE:
        _CACHE["nc"] = _build()
    return _CACHE["nc"]


def _host_inputs(z, rotation_predictions, labels):
    import ml_dtypes

    z = np.ascontiguousarray(np.asarray(z, dtype=np.float32))
    rp = np.ascontiguousarray(np.asarray(rotation_predictions, dtype=np.float32))
    lab = np.asarray(labels).astype(np.int64)
    oh_full = np.eye(4, dtype=np.float32)[lab % 4]

    idm = np.eye(128, dtype=np.float32)
    idmb = np.eye(128, dtype=ml_dtypes.bfloat16)
    pidx = np.arange(128)
    pmk = np.zeros((128, 128), dtype=np.float32)
    pmk[pidx, pidx ^ 1] = 1.0

    in_maps = []
    for c in range(N_CORES):
        r0, r1 = c * SLAB, (c + 1) * SLAB
        in_maps.append(
            {
                "z": z,
                "zslab": z[r0:r1],
                "rp": rp[r0:r1],
                "oh": oh_full[r0:r1],
                "idm": idm,
                "idmb": idmb,
                "pm": pmk,
            }
        )
    return in_maps


def kernel(z, rotation_predictions, labels):
    nc = get_nc()
    in_maps = _host_inputs(z, rotation_predictions, labels)
    res = run_bass_kernel_spmd(nc, in_maps, core_ids=list(range(N_CORES)))
    total = sum(float(res.results[c]["partial"][0, 0]) for c in range(N_CORES))
    return np.float32(total / B)


if __name__ == "__main__":
    rng = np.random.default_rng(0)
    z = rng.standard_normal((B, D), dtype=np.float32)
    rp = rng.standard_normal((B, 4), dtype=np.float32)
    lab = rng.integers(0, 4, size=(B,)).astype(np.int64)
    print("loss:", kernel(z, rp, lab))


# revision 18
# speedup vs baseline: 1.4539x; 1.0110x over previous
"""CSILoss (contrastive + rotation CE) Trainium2 kernel.

Contract: kernel(**inputs) takes the FULL unsharded inputs
  z: [8192, 256] f32, rotation_predictions: [8192, 4] f32, labels: [8192] i64
and returns the full scalar loss (f32), computed on 8 NeuronCores.

Sharding: data-parallel over rows of z. Each core receives the full z (to
build the normalized-transposed embedding matrix znT used as the matmul RHS)
plus its own 1024-row slab (LHS source, rotation slab, label one-hots). Each
core computes its 1024x8192 cosine-similarity slab on the PE (fp8 DoubleRow),
exponentiates with fused row-sum accumulation on the scalar engine, extracts
the positive/diagonal terms from bitwise-identical recomputed diagonal
blocks, and reduces to one scalar partial; the host sums the 8 partials.

Engine split: GpSimd converts z to bf16; DVE computes row sums-of-squares
(fused mul+accum), applies rnorm during a bf16 rescale, copies PSUM->fp8 and
extracts masked terms; PE transposes via identity matmuls and runs the fp8
logits matmuls; the scalar engine does Exp/Ln only (table loads grouped),
with fused row-sum accumulation on the big exponentials. Next-chunk
transposes are interleaved between the exp slots to keep ACT saturated.
"""

import sys

for _p in ("/opt/trn_rl_repo", "/root/.axon_site/_ro/trn_rl_repo"):
    if _p not in sys.path:
        sys.path.insert(0, _p)

import numpy as np

import concourse.bass as bass
import concourse.tile as tile
from concourse import bacc, mybir
from concourse.bass import ds, ts
from concourse.bass_utils import run_bass_kernel_spmd

B, D = 8192, 256
N_CORES = 8
SLAB = B // N_CORES
RB = SLAB // 128
TB = B // 128
F32 = mybir.dt.float32
BF16 = mybir.dt.bfloat16
FP8 = mybir.dt.float8e4
AF = mybir.ActivationFunctionType
ALU = mybir.AluOpType
DR = mybir.MatmulPerfMode.DoubleRow

_CACHE = {}


def _build():
    nc = bacc.Bacc("TRN2", target_bir_lowering=False, debug=False)

    z = nc.declare_dram_parameter("z", [B, D], F32, isOutput=False)
    zslab = nc.declare_dram_parameter("zslab", [SLAB, D], F32, isOutput=False)
    rp = nc.declare_dram_parameter("rp", [SLAB, 4], F32, isOutput=False)
    oh = nc.declare_dram_parameter("oh", [SLAB, 4], F32, isOutput=False)
    idm = nc.declare_dram_parameter("idm", [128, 128], F32, isOutput=False)
    idmb = nc.declare_dram_parameter("idmb", [128, 128], BF16, isOutput=False)
    pm = nc.declare_dram_parameter("pm", [128, 128], F32, isOutput=False)
    partial = nc.declare_dram_parameter("partial", [1, 1], F32, isOutput=True)

    with tile.TileContext(nc) as tc:
        from contextlib import ExitStack

        with ExitStack() as stk:
            const = stk.enter_context(tc.tile_pool(name="const", bufs=1))
            small = stk.enter_context(tc.tile_pool(name="small", bufs=1))
            escp = stk.enter_context(tc.tile_pool(name="esc", bufs=2))
            zf32 = stk.enter_context(tc.tile_pool(name="zf32", bufs=4))
            zbfp = stk.enter_context(tc.tile_pool(name="zbfp", bufs=9))
            znp = stk.enter_context(tc.tile_pool(name="znp", bufs=9))
            sqp = stk.enter_context(tc.tile_pool(name="sqp", bufs=4))
            msc = stk.enter_context(tc.tile_pool(name="msc", bufs=2))
            psp = stk.enter_context(tc.tile_pool(name="psp", bufs=2, space="PSUM"))

            # ---- constants / small inputs
            idm_sb = const.tile([128, 128], F32)
            nc.sync.dma_start(out=idm_sb[:], in_=idm[:])
            idmb_sb = const.tile([128, 128], BF16)
            nc.sync.dma_start(out=idmb_sb[:], in_=idmb[:])
            pm_sb = const.tile([128, 128], F32)
            nc.sync.dma_start(out=pm_sb[:], in_=pm[:])
            rp_sb = const.tile([128, RB, 4], F32)
            nc.sync.dma_start(out=rp_sb[:], in_=rp[:, :].rearrange("(b p) f -> p b f", p=128))
            oh_sb = const.tile([128, RB, 4], F32)
            nc.sync.dma_start(out=oh_sb[:], in_=oh[:, :].rearrange("(b p) f -> p b f", p=128))
            ones = const.tile([128, 1], F32)
            nc.vector.memset(ones[:], 1.0)

            znT8 = const.tile([128, 2, B], FP8, tag="znT8")
            zsT8 = const.tile([128, 2, SLAB], FP8, tag="zsT8")

            sumsq = small.tile([128, TB], F32)
            rnorm = small.tile([128, TB], F32)
            sumsq_s = small.tile([128, RB], F32)
            rnorm_s = small.tile([128, RB], F32)
            posv = small.tile([128, RB], F32)
            diagv = small.tile([128, RB], F32)
            acc = small.tile([128, RB, 4], F32)

            def sumsq_of(dst_col, src_ap):
                scr = sqp.tile([128, D], BF16, tag="sqscr")
                nc.vector.scalar_tensor_tensor(
                    out=scr[:], in0=src_ap, scalar=1.0, in1=src_ap,
                    op0=ALU.mult, op1=ALU.mult, accum_out=dst_col,
                )

            def rsqrt_of(dst_sl, src_sl, k):
                # dst = min(rsqrt(src), 1e8) entirely on DVE:
                # Quake-III seed + 2 Newton iterations (rel err ~3e-7).
                sb = src_sl.bitcast(mybir.dt.uint32)
                hbits = sqp.tile([128, k], mybir.dt.int32, tag=f"rsq_h{k}")
                nc.vector.tensor_scalar(
                    out=hbits[:].bitcast(mybir.dt.uint32), in0=sb, scalar1=1,
                    scalar2=None, op0=ALU.logical_shift_right,
                )
                seed = sqp.tile([128, k], mybir.dt.int32, tag=f"rsq_s{k}")
                nc.vector.tensor_scalar(
                    out=seed[:], in0=hbits[:], scalar1=-1, scalar2=0x5F3759DF,
                    op0=ALU.mult, op1=ALU.add,
                )
                y = seed[:].bitcast(F32)
                y2 = sqp.tile([128, k], F32, tag=f"rsq_y2{k}")
                w = sqp.tile([128, k], F32, tag=f"rsq_w{k}")
                for _ in range(2):
                    nc.vector.tensor_tensor(out=y2[:], in0=y, in1=y, op=ALU.mult)
                    nc.vector.scalar_tensor_tensor(
                        out=w[:], in0=y2[:], scalar=-0.5, in1=src_sl,
                        op0=ALU.mult, op1=ALU.mult,
                    )
                    nc.vector.tensor_scalar(
                        out=w[:], in0=w[:], scalar1=1.5, scalar2=None, op0=ALU.add
                    )
                    nc.vector.tensor_tensor(out=y, in0=y, in1=w[:], op=ALU.mult)
                nc.vector.tensor_scalar(
                    out=dst_sl, in0=y, scalar1=1e8, scalar2=None, op0=ALU.min
                )

            # normalized bf16 z per chunk (natural layout), block t scaled by rnorm_t
            znbf = [None] * 8

            def load_chunk(g):
                zf = zf32.tile([128, 8, D], F32, tag="zf", name=f"zf{g}")
                nc.sync.dma_start(
                    out=zf[:],
                    in_=z[g * 1024 : (g + 1) * 1024, :].rearrange(
                        "(b p) d -> p b d", p=128
                    ),
                )
                zb = zbfp.tile([128, 8, D], BF16, tag="zbf", name=f"zbf{g}")
                nc.gpsimd.tensor_copy(zb[:], zf[:])
                for b in range(8):
                    t = 8 * g + b
                    sumsq_of(sumsq[:, t : t + 1], zb[:, b, :])
                znbf[g] = zb

            def scale_chunk(g):
                # rescale in place is not allowed across engines; write to znp tile
                zn = znp.tile([128, 8, D], BF16, tag="znbf", name=f"znbf{g}")
                for b in range(8):
                    t = 8 * g + b
                    nc.vector.tensor_scalar_mul(
                        out=zn[:, b, :], in0=znbf[g][:, b, :], scalar1=rnorm[:, t : t + 1]
                    )
                znbf[g] = zn

            # ---- slab + chunks 0..3: load, convert, sumsq
            zs_f = zf32.tile([128, RB, D], F32, tag="zf")
            nc.sync.dma_start(
                out=zs_f[:], in_=zslab[:, :].rearrange("(b p) d -> p b d", p=128)
            )
            zs_b = zbfp.tile([128, RB, D], BF16, tag="zbf")
            nc.vector.tensor_copy(zs_b[:], zs_f[:])
            for b in range(RB):
                sumsq_of(sumsq_s[:, b : b + 1], zs_b[:, b, :])
            rsqrt_of(rnorm_s[:, :], sumsq_s[:, :], RB)
            for g in range(4):
                load_chunk(g)
            rsqrt_of(rnorm[:, 0:16], sumsq[:, 0:16], 16)
            rsqrt_of(rnorm[:, 16:32], sumsq[:, 16:32], 16)

            # rotation exps early (Exp table loaded; ACT otherwise idle)
            rs = small.tile([128, RB], F32)
            rescr = small.tile([128, RB, 4], F32)
            for b in range(RB):
                nc.scalar.activation(
                    out=rescr[:, b, :],
                    in_=rp_sb[:, b, :],
                    func=AF.Exp,
                    accum_out=rs[:, b : b + 1],
                )

            # ---- slab: rescale, transpose, diagonal blocks, extractions
            zn_s = znp.tile([128, RB, D], BF16, tag="znbf")
            for b in range(RB):
                nc.vector.tensor_scalar_mul(
                    out=zn_s[:, b, :], in0=zs_b[:, b, :], scalar1=rnorm_s[:, b : b + 1]
                )
            ps_s = psp.tile([128, 2048], F32, tag="ps")
            for i in range(RB):
                for h in range(2):
                    nc.tensor.matmul(
                        ps_s[:, ds(h * 1024 + i * 128, 128)],
                        lhsT=zn_s[:, i, ds(h * 128, 128)],
                        rhs=idmb_sb[:],
                        start=True,
                        stop=True,
                    )
            for h in range(2):
                nc.vector.tensor_copy(zsT8[:, h, :], ps_s[:, ds(h * 1024, 1024)])

            ps_d = psp.tile([128, 2048], F32, tag="ps")
            for rb in range(RB):
                nc.tensor.matmul(
                    ps_d[:, ts(rb, 128)],
                    lhsT=zsT8[:, :, ts(rb, 128)],
                    rhs=zsT8[:, :, ts(rb, 128)],
                    start=True,
                    stop=True,
                    perf_mode=DR,
                )
            dcp = const.tile([128, RB, 128], F32)
            nc.vector.tensor_copy(dcp[:], ps_d[:, 0:1024].rearrange("p (i c) -> p i c", c=128))
            for rb in range(RB):
                mscr = msc.tile([128, 128], F32, tag="mscr")
                nc.vector.scalar_tensor_tensor(
                    out=mscr[:], in0=dcp[:, rb, :], scalar=1.0, in1=pm_sb[:],
                    op0=ALU.mult, op1=ALU.mult, accum_out=posv[:, rb : rb + 1],
                )
                mscr2 = msc.tile([128, 128], F32, tag="mscr")
                nc.vector.scalar_tensor_tensor(
                    out=mscr2[:], in0=dcp[:, rb, :], scalar=1.0, in1=idm_sb[:],
                    op0=ALU.mult, op1=ALU.mult, accum_out=diagv[:, rb : rb + 1],
                )
            ed = small.tile([128, RB], F32)
            nc.scalar.activation(out=ed[:], in_=diagv[:], func=AF.Exp, scale=4.0)

            # ---- transpose emission helper (chunk n covers blocks 16n..16n+15)
            def emit_T_half(n, h):
                ps_t = psp.tile([128, 2048], F32, tag="ps", name=f"ps_t{n}_{h}")
                for i in range(16):
                    t = 16 * n + i
                    g, b = divmod(t, 8)
                    nc.tensor.matmul(
                        ps_t[:, ts(i, 128)],
                        lhsT=znbf[g][:, b, ds(h * 128, 128)],
                        rhs=idmb_sb[:],
                        start=True,
                        stop=True,
                    )
                nc.vector.tensor_copy(znT8[:, h, ds(2048 * n, 2048)], ps_t[:])

            scale_chunk(0)
            scale_chunk(1)
            emit_T_half(0, 0)
            emit_T_half(0, 1)

            # ---- streamed chunks: big matmuls + exp; next chunk's transposes
            # interleaved between exp slots
            for n in range(4):
                for rb in range(RB):
                    ps = psp.tile([128, 2048], F32, tag="ps")
                    for s in range(4):
                        nc.tensor.matmul(
                            ps[:, ts(s, 512)],
                            lhsT=zsT8[:, :, ts(rb, 128)],
                            rhs=znT8[:, :, ds(2048 * n + 512 * s, 512)],
                            start=True,
                            stop=True,
                            perf_mode=DR,
                        )
                    e = escp.tile([128, 2048], BF16, tag="esc")
                    nc.scalar.activation(
                        out=e[:],
                        in_=ps[:],
                        func=AF.Exp,
                        scale=4.0,
                        accum_out=acc[:, rb, n : n + 1],
                    )
                    if n < 3 and rb == 3:
                        scale_chunk(2 * (n + 1))
                        scale_chunk(2 * (n + 1) + 1)
                    if n < 3 and rb == 4:
                        emit_T_half(n + 1, 0)
                    if n < 3 and rb == 6:
                        emit_T_half(n + 1, 1)
                if n == 0:
                    for g in range(4, 8):
                        load_chunk(g)
                    rsqrt_of(rnorm[:, 32:48], sumsq[:, 32:48], 16)
                    rsqrt_of(rnorm[:, 48:64], sumsq[:, 48:64], 16)

            # ---- finals (Ln ops grouped)
            S = small.tile([128, RB], F32)
            nc.vector.reduce_sum(S[:], acc[:], axis=mybir.AxisListType.X)
            Sm = small.tile([128, RB], F32)
            nc.vector.tensor_tensor(out=Sm[:], in0=S[:], in1=ed[:], op=ALU.subtract)
            lse = small.tile([128, RB], F32)
            nc.scalar.activation(out=lse[:], in_=Sm[:], func=AF.Ln)
            rlse = small.tile([128, RB], F32)
            nc.scalar.activation(out=rlse[:], in_=rs[:], func=AF.Ln)

            p4 = small.tile([128, RB], F32)
            nc.vector.tensor_scalar_mul(out=p4[:], in0=posv[:], scalar1=4.0)
            lc = small.tile([128, RB], F32)
            nc.vector.tensor_tensor(out=lc[:], in0=lse[:], in1=p4[:], op=ALU.subtract)
            picked = small.tile([128, 1], F32)
            pscr = small.tile([128, RB, 4], F32)
            nc.vector.scalar_tensor_tensor(
                out=pscr[:], in0=rp_sb[:], scalar=1.0, in1=oh_sb[:],
                op0=ALU.mult, op1=ALU.mult, accum_out=picked[:],
            )
            csum = small.tile([128, 1], F32)
            nc.vector.reduce_sum(csum[:], lc[:], axis=mybir.AxisListType.X)
            rsum = small.tile([128, 1], F32)
            nc.vector.reduce_sum(rsum[:], rlse[:], axis=mybir.AxisListType.X)
            tot = small.tile([128, 1], F32)
            nc.vector.tensor_tensor(out=tot[:], in0=csum[:], in1=rsum[:], op=ALU.add)
            nc.vector.tensor_tensor(out=tot[:], in0=tot[:], in1=picked[:], op=ALU.subtract)

            psF = psp.tile([128, 2048], F32, tag="ps")
            nc.tensor.matmul(psF[0:1, 0:1], lhsT=tot[:], rhs=ones[:], start=True, stop=True)
            outsb = small.tile([1, 1], F32)
            nc.vector.tensor_copy(outsb[:], psF[0:1, 0:1])
            nc.sync.dma_start(out=partial[:], in_=outsb[:])

    nc.compile()
    return nc


def get_nc():
    if "nc" not in _CACHE:
        _CACHE["nc"] = _build()
    return _CACHE["nc"]


def _host_inputs(z, rotation_predictions, labels):
    import ml_dtypes

    z = np.ascontiguousarray(np.asarray(z, dtype=np.float32))
    rp = np.ascontiguousarray(np.asarray(rotation_predictions, dtype=np.float32))
    lab = np.asarray(labels).astype(np.int64)
    oh_full = np.eye(4, dtype=np.float32)[lab % 4]

    idm = np.eye(128, dtype=np.float32)
    idmb = np.eye(128, dtype=ml_dtypes.bfloat16)
    pidx = np.arange(128)
    pmk = np.zeros((128, 128), dtype=np.float32)
    pmk[pidx, pidx ^ 1] = 1.0

    in_maps = []
    for c in range(N_CORES):
        r0, r1 = c * SLAB, (c + 1) * SLAB
        in_maps.append(
            {
                "z": z,
                "zslab": z[r0:r1],
                "rp": rp[r0:r1],
                "oh": oh_full[r0:r1],
                "idm": idm,
                "idmb": idmb,
                "pm": pmk,
            }
        )
    return in_maps


def kernel(z, rotation_predictions, labels):
    nc = get_nc()
    in_maps = _host_inputs(z, rotation_predictions, labels)
    res = run_bass_kernel_spmd(nc, in_maps, core_ids=list(range(N_CORES)))
    total = sum(float(res.results[c]["partial"][0, 0]) for c in range(N_CORES))
    return np.float32(total / B)


if __name__ == "__main__":
    rng = np.random.default_rng(0)
    z = rng.standard_normal((B, D), dtype=np.float32)
    rp = rng.standard_normal((B, 4), dtype=np.float32)
    lab = rng.integers(0, 4, size=(B,)).astype(np.int64)
    print("loss:", kernel(z, rp, lab))


# revision 21
# speedup vs baseline: 1.4907x; 1.0253x over previous
"""CSILoss (contrastive + rotation CE) Trainium2 kernel.

Contract: kernel(**inputs) takes the FULL unsharded inputs
  z: [8192, 256] f32, rotation_predictions: [8192, 4] f32, labels: [8192] i64
and returns the full scalar loss (f32), computed on 8 NeuronCores.

Sharding: data-parallel over rows of z. Each core receives the full z (to
build the normalized-transposed embedding matrix znT used as the matmul RHS)
plus its own 1024-row slab (LHS source, rotation slab, label one-hots). Each
core computes its 1024x8192 cosine-similarity slab on the PE (fp8 DoubleRow),
exponentiates with fused row-sum accumulation on the scalar engine, extracts
the positive/diagonal terms from bitwise-identical recomputed diagonal
blocks, and reduces to one scalar partial; the host sums the 8 partials.

Engine split: GpSimd converts z to bf16; DVE computes row sums-of-squares
(fused mul+accum), applies rnorm during a bf16 rescale, copies PSUM->fp8 and
extracts masked terms; PE transposes via identity matmuls and runs the fp8
logits matmuls; the scalar engine does Exp/Ln only (table loads grouped),
with fused row-sum accumulation on the big exponentials. Next-chunk
transposes are interleaved between the exp slots to keep ACT saturated.
"""

import sys

for _p in ("/opt/trn_rl_repo", "/root/.axon_site/_ro/trn_rl_repo"):
    if _p not in sys.path:
        sys.path.insert(0, _p)

import numpy as np

import concourse.bass as bass
import concourse.tile as tile
from concourse import bacc, mybir
from concourse.bass import ds, ts
from concourse.bass_utils import run_bass_kernel_spmd

B, D = 8192, 256
N_CORES = 8
SLAB = B // N_CORES
RB = SLAB // 128
TB = B // 128
F32 = mybir.dt.float32
BF16 = mybir.dt.bfloat16
FP8 = mybir.dt.float8e4
AF = mybir.ActivationFunctionType
ALU = mybir.AluOpType
DR = mybir.MatmulPerfMode.DoubleRow

_CACHE = {}


def _build():
    nc = bacc.Bacc("TRN2", target_bir_lowering=False, debug=False)

    z = nc.declare_dram_parameter("z", [B, D], F32, isOutput=False)
    zslab = nc.declare_dram_parameter("zslab", [SLAB, D], F32, isOutput=False)
    rp = nc.declare_dram_parameter("rp", [SLAB, 4], F32, isOutput=False)
    oh = nc.declare_dram_parameter("oh", [SLAB, 4], F32, isOutput=False)
    idm = nc.declare_dram_parameter("idm", [128, 128], F32, isOutput=False)
    idmb = nc.declare_dram_parameter("idmb", [128, 128], BF16, isOutput=False)
    pm = nc.declare_dram_parameter("pm", [128, 128], F32, isOutput=False)
    partial = nc.declare_dram_parameter("partial", [1, 1], F32, isOutput=True)

    with tile.TileContext(nc) as tc:
        from contextlib import ExitStack

        with ExitStack() as stk:
            const = stk.enter_context(tc.tile_pool(name="const", bufs=1))
            small = stk.enter_context(tc.tile_pool(name="small", bufs=1))
            escp = stk.enter_context(tc.tile_pool(name="esc", bufs=2))
            zf32 = stk.enter_context(tc.tile_pool(name="zf32", bufs=4))
            zbfp = stk.enter_context(tc.tile_pool(name="zbfp", bufs=9))
            znp = stk.enter_context(tc.tile_pool(name="znp", bufs=9))
            sqp = stk.enter_context(tc.tile_pool(name="sqp", bufs=4))
            msc = stk.enter_context(tc.tile_pool(name="msc", bufs=2))
            psp = stk.enter_context(tc.tile_pool(name="psp", bufs=2, space="PSUM"))

            # ---- constants / small inputs
            idm_sb = const.tile([128, 128], F32)
            nc.sync.dma_start(out=idm_sb[:], in_=idm[:])
            idmb_sb = const.tile([128, 128], BF16)
            nc.sync.dma_start(out=idmb_sb[:], in_=idmb[:])
            pm_sb = const.tile([128, 128], F32)
            nc.sync.dma_start(out=pm_sb[:], in_=pm[:])
            rp_sb = const.tile([128, RB, 4], F32)
            nc.sync.dma_start(out=rp_sb[:], in_=rp[:, :].rearrange("(b p) f -> p b f", p=128))
            oh_sb = const.tile([128, RB, 4], F32)
            nc.sync.dma_start(out=oh_sb[:], in_=oh[:, :].rearrange("(b p) f -> p b f", p=128))
            ones = const.tile([128, 1], F32)
            nc.vector.memset(ones[:], 1.0)

            znT8 = const.tile([128, 2, B], FP8, tag="znT8")
            zsT8 = const.tile([128, 2, SLAB], FP8, tag="zsT8")

            sumsq = small.tile([128, TB], F32)
            rnorm = small.tile([128, TB], F32)
            sumsq_s = small.tile([128, RB], F32)
            rnorm_s = small.tile([128, RB], F32)
            posv = small.tile([128, RB], F32)
            diagv = small.tile([128, RB], F32)
            acc = small.tile([128, RB, 4], F32)

            def sumsq_of(dst_col, src_ap):
                scr = sqp.tile([128, D], BF16, tag="sqscr")
                nc.vector.scalar_tensor_tensor(
                    out=scr[:], in0=src_ap, scalar=1.0, in1=src_ap,
                    op0=ALU.mult, op1=ALU.mult, accum_out=dst_col,
                )

            def rsqrt_of(dst_sl, src_sl, k):
                # dst = min(rsqrt(src), 1e8) entirely on DVE:
                # Quake-III seed + 2 Newton iterations (rel err ~3e-7).
                sb = src_sl.bitcast(mybir.dt.uint32)
                hbits = sqp.tile([128, k], mybir.dt.int32, tag=f"rsq_h{k}")
                nc.vector.tensor_scalar(
                    out=hbits[:].bitcast(mybir.dt.uint32), in0=sb, scalar1=1,
                    scalar2=None, op0=ALU.logical_shift_right,
                )
                seed = sqp.tile([128, k], mybir.dt.int32, tag=f"rsq_s{k}")
                nc.vector.tensor_scalar(
                    out=seed[:], in0=hbits[:], scalar1=-1, scalar2=0x5F3759DF,
                    op0=ALU.mult, op1=ALU.add,
                )
                y = seed[:].bitcast(F32)
                y2 = sqp.tile([128, k], F32, tag=f"rsq_y2{k}")
                w = sqp.tile([128, k], F32, tag=f"rsq_w{k}")
                for _ in range(2):
                    nc.vector.tensor_tensor(out=y2[:], in0=y, in1=y, op=ALU.mult)
                    nc.vector.scalar_tensor_tensor(
                        out=w[:], in0=y2[:], scalar=-0.5, in1=src_sl,
                        op0=ALU.mult, op1=ALU.mult,
                    )
                    nc.vector.tensor_scalar(
                        out=w[:], in0=w[:], scalar1=1.5, scalar2=None, op0=ALU.add
                    )
                    nc.vector.tensor_tensor(out=y, in0=y, in1=w[:], op=ALU.mult)
                nc.vector.tensor_scalar(
                    out=dst_sl, in0=y, scalar1=1e8, scalar2=None, op0=ALU.min
                )

            # normalized bf16 z per chunk (natural layout), block t scaled by rnorm_t
            znbf = [None] * 8

            def load_chunk(g):
                zf = zf32.tile([128, 8, D], F32, tag="zf", name=f"zf{g}")
                nc.sync.dma_start(
                    out=zf[:],
                    in_=z[g * 1024 : (g + 1) * 1024, :].rearrange(
                        "(b p) d -> p b d", p=128
                    ),
                )
                zb = zbfp.tile([128, 8, D], BF16, tag="zbf", name=f"zbf{g}")
                nc.gpsimd.tensor_copy(zb[:], zf[:])
                for b in range(8):
                    t = 8 * g + b
                    sumsq_of(sumsq[:, t : t + 1], zb[:, b, :])
                znbf[g] = zb

            def scale_chunk(g):
                # rescale in place is not allowed across engines; write to znp tile
                zn = znp.tile([128, 8, D], BF16, tag="znbf", name=f"znbf{g}")
                for b in range(8):
                    t = 8 * g + b
                    nc.vector.tensor_scalar_mul(
                        out=zn[:, b, :], in0=znbf[g][:, b, :], scalar1=rnorm[:, t : t + 1]
                    )
                znbf[g] = zn

            # ---- slab + chunks 0..3: load, convert, sumsq
            zs_f = zf32.tile([128, RB, D], F32, tag="zf")
            nc.sync.dma_start(
                out=zs_f[:], in_=zslab[:, :].rearrange("(b p) d -> p b d", p=128)
            )
            zs_b = zbfp.tile([128, RB, D], BF16, tag="zbf")
            nc.vector.tensor_copy(zs_b[:], zs_f[:])
            for b in range(RB):
                sumsq_of(sumsq_s[:, b : b + 1], zs_b[:, b, :])
            rsqrt_of(rnorm_s[:, :], sumsq_s[:, :], RB)

            # ---- slab: rescale, transpose, diagonal blocks, extractions
            zn_s = znp.tile([128, RB, D], BF16, tag="znbf")
            for b in range(RB):
                nc.vector.tensor_scalar_mul(
                    out=zn_s[:, b, :], in0=zs_b[:, b, :], scalar1=rnorm_s[:, b : b + 1]
                )
            ps_s = psp.tile([128, 2048], F32, tag="ps")
            for i in range(RB):
                for h in range(2):
                    nc.tensor.matmul(
                        ps_s[:, ds(h * 1024 + i * 128, 128)],
                        lhsT=zn_s[:, i, ds(h * 128, 128)],
                        rhs=idmb_sb[:],
                        start=True,
                        stop=True,
                    )
            for h in range(2):
                nc.vector.tensor_copy(zsT8[:, h, :], ps_s[:, ds(h * 1024, 1024)])

            ps_d = psp.tile([128, 2048], F32, tag="ps")
            for rb in range(RB):
                nc.tensor.matmul(
                    ps_d[:, ts(rb, 128)],
                    lhsT=zsT8[:, :, ts(rb, 128)],
                    rhs=zsT8[:, :, ts(rb, 128)],
                    start=True,
                    stop=True,
                    perf_mode=DR,
                )
            dcp = const.tile([128, RB, 128], F32)
            nc.vector.tensor_copy(dcp[:], ps_d[:, 0:1024].rearrange("p (i c) -> p i c", c=128))
            for rb in range(RB):
                mscr = msc.tile([128, 128], F32, tag="mscr")
                nc.vector.scalar_tensor_tensor(
                    out=mscr[:], in0=dcp[:, rb, :], scalar=1.0, in1=pm_sb[:],
                    op0=ALU.mult, op1=ALU.mult, accum_out=posv[:, rb : rb + 1],
                )
                mscr2 = msc.tile([128, 128], F32, tag="mscr")
                nc.vector.scalar_tensor_tensor(
                    out=mscr2[:], in0=dcp[:, rb, :], scalar=1.0, in1=idm_sb[:],
                    op0=ALU.mult, op1=ALU.mult, accum_out=diagv[:, rb : rb + 1],
                )
            ed = small.tile([128, RB], F32)
            nc.scalar.activation(out=ed[:], in_=diagv[:], func=AF.Exp, scale=4.0)

            # rotation exps early (ACT otherwise idle in the lead-in)
            rs = small.tile([128, RB], F32)
            rescr = small.tile([128, RB, 4], F32)
            for b in range(RB):
                nc.scalar.activation(
                    out=rescr[:, b, :],
                    in_=rp_sb[:, b, :],
                    func=AF.Exp,
                    accum_out=rs[:, b : b + 1],
                )

            # first two chunks -> rnorm(0:16) gates chunk-0 transposes
            load_chunk(0)
            load_chunk(1)
            rsqrt_of(rnorm[:, 0:16], sumsq[:, 0:16], 16)

            # ---- transpose emission helper (chunk n covers blocks 16n..16n+15)
            def emit_T_half(n, h):
                ps_t = psp.tile([128, 2048], F32, tag="ps", name=f"ps_t{n}_{h}")
                for i in range(16):
                    t = 16 * n + i
                    g, b = divmod(t, 8)
                    nc.tensor.matmul(
                        ps_t[:, ts(i, 128)],
                        lhsT=znbf[g][:, b, ds(h * 128, 128)],
                        rhs=idmb_sb[:],
                        start=True,
                        stop=True,
                    )
                nc.vector.tensor_copy(znT8[:, h, ds(2048 * n, 2048)], ps_t[:])

            scale_chunk(0)
            scale_chunk(1)
            emit_T_half(0, 0)
            emit_T_half(0, 1)
            load_chunk(2)
            load_chunk(3)
            rsqrt_of(rnorm[:, 16:32], sumsq[:, 16:32], 16)

            # ---- streamed chunks: big matmuls + exp; next chunk's transposes
            # interleaved between exp slots
            for n in range(4):
                for rb in range(RB):
                    ps = psp.tile([128, 2048], F32, tag="ps")
                    for s in range(4):
                        nc.tensor.matmul(
                            ps[:, ts(s, 512)],
                            lhsT=zsT8[:, :, ts(rb, 128)],
                            rhs=znT8[:, :, ds(2048 * n + 512 * s, 512)],
                            start=True,
                            stop=True,
                            perf_mode=DR,
                        )
                    e = escp.tile([128, 2048], BF16, tag="esc")
                    nc.scalar.activation(
                        out=e[:],
                        in_=ps[:],
                        func=AF.Exp,
                        scale=4.0,
                        accum_out=acc[:, rb, n : n + 1],
                    )
                    if n < 3 and rb == 3:
                        scale_chunk(2 * (n + 1))
                        scale_chunk(2 * (n + 1) + 1)
                    if n < 3 and rb == 4:
                        emit_T_half(n + 1, 0)
                    if n < 3 and rb == 6:
                        emit_T_half(n + 1, 1)
                if n == 0:
                    for g in range(4, 8):
                        load_chunk(g)
                    rsqrt_of(rnorm[:, 32:48], sumsq[:, 32:48], 16)
                    rsqrt_of(rnorm[:, 48:64], sumsq[:, 48:64], 16)

            # ---- finals (Ln ops grouped)
            S = small.tile([128, RB], F32)
            nc.vector.reduce_sum(S[:], acc[:], axis=mybir.AxisListType.X)
            Sm = small.tile([128, RB], F32)
            nc.vector.tensor_tensor(out=Sm[:], in0=S[:], in1=ed[:], op=ALU.subtract)
            lse = small.tile([128, RB], F32)
            nc.scalar.activation(out=lse[:], in_=Sm[:], func=AF.Ln)
            rlse = small.tile([128, RB], F32)
            nc.scalar.activation(out=rlse[:], in_=rs[:], func=AF.Ln)

            p4 = small.tile([128, RB], F32)
            nc.vector.tensor_scalar_mul(out=p4[:], in0=posv[:], scalar1=4.0)
            lc = small.tile([128, RB], F32)
            nc.vector.tensor_tensor(out=lc[:], in0=lse[:], in1=p4[:], op=ALU.subtract)
            picked = small.tile([128, 1], F32)
            pscr = small.tile([128, RB, 4], F32)
            nc.vector.scalar_tensor_tensor(
                out=pscr[:], in0=rp_sb[:], scalar=1.0, in1=oh_sb[:],
                op0=ALU.mult, op1=ALU.mult, accum_out=picked[:],
            )
            csum = small.tile([128, 1], F32)
            nc.vector.reduce_sum(csum[:], lc[:], axis=mybir.AxisListType.X)
            rsum = small.tile([128, 1], F32)
            nc.vector.reduce_sum(rsum[:], rlse[:], axis=mybir.AxisListType.X)
            tot = small.tile([128, 1], F32)
            nc.vector.tensor_tensor(out=tot[:], in0=csum[:], in1=rsum[:], op=ALU.add)
            nc.vector.tensor_tensor(out=tot[:], in0=tot[:], in1=picked[:], op=ALU.subtract)

            psF = psp.tile([128, 2048], F32, tag="ps")
            nc.tensor.matmul(psF[0:1, 0:1], lhsT=tot[:], rhs=ones[:], start=True, stop=True)
            outsb = small.tile([1, 1], F32)
            nc.vector.tensor_copy(outsb[:], psF[0:1, 0:1])
            nc.sync.dma_start(out=partial[:], in_=outsb[:])

    nc.compile()
    return nc


def get_nc():
    if "nc" not in _CACHE:
        _CACHE["nc"] = _build()
    return _CACHE["nc"]


def _host_inputs(z, rotation_predictions, labels):
    import ml_dtypes

    z = np.ascontiguousarray(np.asarray(z, dtype=np.float32))
    rp = np.ascontiguousarray(np.asarray(rotation_predictions, dtype=np.float32))
    lab = np.asarray(labels).astype(np.int64)
    oh_full = np.eye(4, dtype=np.float32)[lab % 4]

    idm = np.eye(128, dtype=np.float32)
    idmb = np.eye(128, dtype=ml_dtypes.bfloat16)
    pidx = np.arange(128)
    pmk = np.zeros((128, 128), dtype=np.float32)
    pmk[pidx, pidx ^ 1] = 1.0

    in_maps = []
    for c in range(N_CORES):
        r0, r1 = c * SLAB, (c + 1) * SLAB
        in_maps.append(
            {
                "z": z,
                "zslab": z[r0:r1],
                "rp": rp[r0:r1],
                "oh": oh_full[r0:r1],
                "idm": idm,
                "idmb": idmb,
                "pm": pmk,
            }
        )
    return in_maps


def kernel(z, rotation_predictions, labels):
    nc = get_nc()
    in_maps = _host_inputs(z, rotation_predictions, labels)
    res = run_bass_kernel_spmd(nc, in_maps, core_ids=list(range(N_CORES)))
    total = sum(float(res.results[c]["partial"][0, 0]) for c in range(N_CORES))
    return np.float32(total / B)


if __name__ == "__main__":
    rng = np.random.default_rng(0)
    z = rng.standard_normal((B, D), dtype=np.float32)
    rp = rng.standard_normal((B, 4), dtype=np.float32)
    lab = rng.integers(0, 4, size=(B,)).astype(np.int64)
    print("loss:", kernel(z, rp, lab))


# revision 22
# speedup vs baseline: 1.5425x; 1.0347x over previous
"""CSILoss (contrastive + rotation CE) Trainium2 kernel.

Contract: kernel(**inputs) takes the FULL unsharded inputs
  z: [8192, 256] f32, rotation_predictions: [8192, 4] f32, labels: [8192] i64
and returns the full scalar loss (f32), computed on 8 NeuronCores.

Sharding: data-parallel over rows of z. Each core receives the full z (to
build the normalized-transposed embedding matrix znT used as the matmul RHS)
plus its own 1024-row slab (LHS source, rotation slab, label one-hots). Each
core computes its 1024x8192 cosine-similarity slab on the PE (fp8 DoubleRow),
exponentiates with fused row-sum accumulation on the scalar engine, extracts
the positive/diagonal terms from bitwise-identical recomputed diagonal
blocks, and reduces to one scalar partial; the host sums the 8 partials.

Engine split: GpSimd converts z to bf16; DVE computes row sums-of-squares
(fused mul+accum), applies rnorm during a bf16 rescale, copies PSUM->fp8 and
extracts masked terms; PE transposes via identity matmuls and runs the fp8
logits matmuls; the scalar engine does Exp/Ln only (table loads grouped),
with fused row-sum accumulation on the big exponentials. Next-chunk
transposes are interleaved between the exp slots to keep ACT saturated.
"""

import sys

for _p in ("/opt/trn_rl_repo", "/root/.axon_site/_ro/trn_rl_repo"):
    if _p not in sys.path:
        sys.path.insert(0, _p)

import numpy as np

import concourse.bass as bass
import concourse.tile as tile
from concourse import bacc, mybir
from concourse.bass import ds, ts
from concourse.bass_utils import run_bass_kernel_spmd

B, D = 8192, 256
N_CORES = 8
SLAB = B // N_CORES
RB = SLAB // 128
TB = B // 128
F32 = mybir.dt.float32
BF16 = mybir.dt.bfloat16
FP8 = mybir.dt.float8e4
AF = mybir.ActivationFunctionType
ALU = mybir.AluOpType
DR = mybir.MatmulPerfMode.DoubleRow

_CACHE = {}


def _build():
    nc = bacc.Bacc("TRN2", target_bir_lowering=False, debug=False)

    z = nc.declare_dram_parameter("z", [B, D], F32, isOutput=False)
    zslab = nc.declare_dram_parameter("zslab", [SLAB, D], F32, isOutput=False)
    rp = nc.declare_dram_parameter("rp", [SLAB, 4], F32, isOutput=False)
    oh = nc.declare_dram_parameter("oh", [SLAB, 4], F32, isOutput=False)
    idm = nc.declare_dram_parameter("idm", [128, 128], F32, isOutput=False)
    idmb = nc.declare_dram_parameter("idmb", [128, 128], BF16, isOutput=False)
    pm = nc.declare_dram_parameter("pm", [128, 128], F32, isOutput=False)
    partial = nc.declare_dram_parameter("partial", [1, 1], F32, isOutput=True)

    with tile.TileContext(nc) as tc:
        from contextlib import ExitStack

        with ExitStack() as stk:
            const = stk.enter_context(tc.tile_pool(name="const", bufs=1))
            small = stk.enter_context(tc.tile_pool(name="small", bufs=1))
            escp = stk.enter_context(tc.tile_pool(name="esc", bufs=2))
            zf32 = stk.enter_context(tc.tile_pool(name="zf32", bufs=4))
            zbfp = stk.enter_context(tc.tile_pool(name="zbfp", bufs=9))
            znp = stk.enter_context(tc.tile_pool(name="znp", bufs=9))
            sqp = stk.enter_context(tc.tile_pool(name="sqp", bufs=4))
            msc = stk.enter_context(tc.tile_pool(name="msc", bufs=2))
            psp = stk.enter_context(tc.tile_pool(name="psp", bufs=2, space="PSUM"))

            # ---- constants / small inputs
            idm_sb = const.tile([128, 128], F32)
            nc.sync.dma_start(out=idm_sb[:], in_=idm[:])
            idmb_sb = const.tile([128, 128], BF16)
            nc.sync.dma_start(out=idmb_sb[:], in_=idmb[:])
            pm_sb = const.tile([128, 128], F32)
            nc.sync.dma_start(out=pm_sb[:], in_=pm[:])
            rp_sb = const.tile([128, RB, 4], F32)
            nc.sync.dma_start(out=rp_sb[:], in_=rp[:, :].rearrange("(b p) f -> p b f", p=128))
            oh_sb = const.tile([128, RB, 4], F32)
            nc.sync.dma_start(out=oh_sb[:], in_=oh[:, :].rearrange("(b p) f -> p b f", p=128))
            ones = const.tile([128, 1], F32)
            nc.vector.memset(ones[:], 1.0)

            znT8 = const.tile([128, 2, B], FP8, tag="znT8")
            zsT8 = const.tile([128, 2, SLAB], FP8, tag="zsT8")

            sumsq = small.tile([128, TB], F32)
            rnorm = small.tile([128, TB], F32)
            sumsq_s = small.tile([128, RB], F32)
            rnorm_s = small.tile([128, RB], F32)
            posv = small.tile([128, RB], F32)
            diagv = small.tile([128, RB], F32)
            acc = small.tile([128, RB, 4], F32)

            def sumsq_of(dst_col, src_ap):
                scr = sqp.tile([128, D], BF16, tag="sqscr")
                nc.vector.scalar_tensor_tensor(
                    out=scr[:], in0=src_ap, scalar=1.0, in1=src_ap,
                    op0=ALU.mult, op1=ALU.mult, accum_out=dst_col,
                )

            def rsqrt_of(dst_sl, src_sl, k):
                # dst = min(rsqrt(src), 1e8) entirely on DVE:
                # Quake-III seed + 2 Newton iterations (rel err ~3e-7).
                sb = src_sl.bitcast(mybir.dt.uint32)
                hbits = sqp.tile([128, k], mybir.dt.int32, tag=f"rsq_h{k}")
                nc.vector.tensor_scalar(
                    out=hbits[:].bitcast(mybir.dt.uint32), in0=sb, scalar1=1,
                    scalar2=None, op0=ALU.logical_shift_right,
                )
                seed = sqp.tile([128, k], mybir.dt.int32, tag=f"rsq_s{k}")
                nc.vector.tensor_scalar(
                    out=seed[:], in0=hbits[:], scalar1=-1, scalar2=0x5F3759DF,
                    op0=ALU.mult, op1=ALU.add,
                )
                y = seed[:].bitcast(F32)
                y2 = sqp.tile([128, k], F32, tag=f"rsq_y2{k}")
                w = sqp.tile([128, k], F32, tag=f"rsq_w{k}")
                for _ in range(2):
                    nc.vector.tensor_tensor(out=y2[:], in0=y, in1=y, op=ALU.mult)
                    nc.vector.scalar_tensor_tensor(
                        out=w[:], in0=y2[:], scalar=-0.5, in1=src_sl,
                        op0=ALU.mult, op1=ALU.mult,
                    )
                    nc.vector.tensor_scalar(
                        out=w[:], in0=w[:], scalar1=1.5, scalar2=None, op0=ALU.add
                    )
                    nc.vector.tensor_tensor(out=y, in0=y, in1=w[:], op=ALU.mult)
                nc.vector.tensor_scalar(
                    out=dst_sl, in0=y, scalar1=1e8, scalar2=None, op0=ALU.min
                )

            # normalized bf16 z per chunk (natural layout), block t scaled by rnorm_t
            znbf = [None] * 8

            def load_chunk(g, cvt="g"):
                zf = zf32.tile([128, 8, D], F32, tag="zf", name=f"zf{g}")
                nc.sync.dma_start(
                    out=zf[:],
                    in_=z[g * 1024 : (g + 1) * 1024, :].rearrange(
                        "(b p) d -> p b d", p=128
                    ),
                )
                zb = zbfp.tile([128, 8, D], BF16, tag="zbf", name=f"zbf{g}")
                (nc.gpsimd if cvt == "g" else nc.vector).tensor_copy(zb[:], zf[:])
                for b in range(8):
                    t = 8 * g + b
                    sumsq_of(sumsq[:, t : t + 1], zb[:, b, :])
                znbf[g] = zb

            def scale_chunk(g):
                # rescale in place is not allowed across engines; write to znp tile
                zn = znp.tile([128, 8, D], BF16, tag="znbf", name=f"znbf{g}")
                for b in range(8):
                    t = 8 * g + b
                    nc.vector.tensor_scalar_mul(
                        out=zn[:, b, :], in0=znbf[g][:, b, :], scalar1=rnorm[:, t : t + 1]
                    )
                znbf[g] = zn

            # ---- slab + chunks 0..3: load, convert, sumsq
            zs_f = zf32.tile([128, RB, D], F32, tag="zf")
            nc.sync.dma_start(
                out=zs_f[:], in_=zslab[:, :].rearrange("(b p) d -> p b d", p=128)
            )
            zs_b = zbfp.tile([128, RB, D], BF16, tag="zbf")
            nc.vector.tensor_copy(zs_b[:], zs_f[:])
            for b in range(RB):
                sumsq_of(sumsq_s[:, b : b + 1], zs_b[:, b, :])
            rsqrt_of(rnorm_s[:, :], sumsq_s[:, :], RB)

            # ---- slab: rescale, transpose, diagonal blocks, extractions
            zn_s = znp.tile([128, RB, D], BF16, tag="znbf")
            for b in range(RB):
                nc.vector.tensor_scalar_mul(
                    out=zn_s[:, b, :], in0=zs_b[:, b, :], scalar1=rnorm_s[:, b : b + 1]
                )
            ps_s = psp.tile([128, 2048], F32, tag="ps")
            for i in range(RB):
                for h in range(2):
                    nc.tensor.matmul(
                        ps_s[:, ds(h * 1024 + i * 128, 128)],
                        lhsT=zn_s[:, i, ds(h * 128, 128)],
                        rhs=idmb_sb[:],
                        start=True,
                        stop=True,
                    )
            for h in range(2):
                nc.vector.tensor_copy(zsT8[:, h, :], ps_s[:, ds(h * 1024, 1024)])

            ed = small.tile([128, RB], F32)
            dcp = const.tile([128, RB, 128], F32)

            def emit_diag_blocks():
                ps_d = psp.tile([128, 2048], F32, tag="ps")
                for rb in range(RB):
                    nc.tensor.matmul(
                        ps_d[:, ts(rb, 128)],
                        lhsT=zsT8[:, :, ts(rb, 128)],
                        rhs=zsT8[:, :, ts(rb, 128)],
                        start=True,
                        stop=True,
                        perf_mode=DR,
                    )
                nc.vector.tensor_copy(
                    dcp[:], ps_d[:, 0:1024].rearrange("p (i c) -> p i c", c=128)
                )
                for rb in range(RB):
                    mscr = msc.tile([128, 128], F32, tag="mscr")
                    nc.vector.scalar_tensor_tensor(
                        out=mscr[:], in0=dcp[:, rb, :], scalar=1.0, in1=pm_sb[:],
                        op0=ALU.mult, op1=ALU.mult, accum_out=posv[:, rb : rb + 1],
                    )
                    mscr2 = msc.tile([128, 128], F32, tag="mscr")
                    nc.vector.scalar_tensor_tensor(
                        out=mscr2[:], in0=dcp[:, rb, :], scalar=1.0, in1=idm_sb[:],
                        op0=ALU.mult, op1=ALU.mult, accum_out=diagv[:, rb : rb + 1],
                    )
                nc.scalar.activation(out=ed[:], in_=diagv[:], func=AF.Exp, scale=4.0)

            # first two chunks -> rnorm(0:16) gates chunk-0 transposes
            load_chunk(0, cvt="v")
            load_chunk(1)
            rsqrt_of(rnorm[:, 0:16], sumsq[:, 0:16], 16)

            # rotation exps early (ACT otherwise idle in the lead-in)
            rs = small.tile([128, RB], F32)
            rescr = small.tile([128, RB, 4], F32)
            for b in range(RB):
                nc.scalar.activation(
                    out=rescr[:, b, :],
                    in_=rp_sb[:, b, :],
                    func=AF.Exp,
                    accum_out=rs[:, b : b + 1],
                )

            # ---- transpose emission helper (chunk n covers blocks 16n..16n+15)
            def emit_T_half(n, h):
                ps_t = psp.tile([128, 2048], F32, tag="ps", name=f"ps_t{n}_{h}")
                for i in range(16):
                    t = 16 * n + i
                    g, b = divmod(t, 8)
                    nc.tensor.matmul(
                        ps_t[:, ts(i, 128)],
                        lhsT=znbf[g][:, b, ds(h * 128, 128)],
                        rhs=idmb_sb[:],
                        start=True,
                        stop=True,
                    )
                nc.vector.tensor_copy(znT8[:, h, ds(2048 * n, 2048)], ps_t[:])

            scale_chunk(0)
            scale_chunk(1)
            emit_T_half(0, 0)
            emit_T_half(0, 1)
            load_chunk(2)
            load_chunk(3)
            rsqrt_of(rnorm[:, 16:32], sumsq[:, 16:32], 16)

            # ---- streamed chunks: big matmuls + exp; next chunk's transposes
            # interleaved between exp slots
            for n in range(4):
                for rb in range(RB):
                    ps = psp.tile([128, 2048], F32, tag="ps")
                    for s in range(4):
                        nc.tensor.matmul(
                            ps[:, ts(s, 512)],
                            lhsT=zsT8[:, :, ts(rb, 128)],
                            rhs=znT8[:, :, ds(2048 * n + 512 * s, 512)],
                            start=True,
                            stop=True,
                            perf_mode=DR,
                        )
                    e = escp.tile([128, 2048], BF16, tag="esc")
                    nc.scalar.activation(
                        out=e[:],
                        in_=ps[:],
                        func=AF.Exp,
                        scale=4.0,
                        accum_out=acc[:, rb, n : n + 1],
                    )
                    if n < 3 and rb == 3:
                        scale_chunk(2 * (n + 1))
                        scale_chunk(2 * (n + 1) + 1)
                    if n < 3 and rb == 4:
                        emit_T_half(n + 1, 0)
                    if n < 3 and rb == 6:
                        emit_T_half(n + 1, 1)
                    if n == 3 and rb == 4:
                        emit_diag_blocks()
                if n == 0:
                    for g in range(4, 8):
                        load_chunk(g)
                    rsqrt_of(rnorm[:, 32:48], sumsq[:, 32:48], 16)
                    rsqrt_of(rnorm[:, 48:64], sumsq[:, 48:64], 16)

            # ---- finals (Ln ops grouped)
            S = small.tile([128, RB], F32)
            nc.vector.reduce_sum(S[:], acc[:], axis=mybir.AxisListType.X)
            Sm = small.tile([128, RB], F32)
            nc.vector.tensor_tensor(out=Sm[:], in0=S[:], in1=ed[:], op=ALU.subtract)
            lse = small.tile([128, RB], F32)
            nc.scalar.activation(out=lse[:], in_=Sm[:], func=AF.Ln)
            rlse = small.tile([128, RB], F32)
            nc.scalar.activation(out=rlse[:], in_=rs[:], func=AF.Ln)

            p4 = small.tile([128, RB], F32)
            nc.vector.tensor_scalar_mul(out=p4[:], in0=posv[:], scalar1=4.0)
            lc = small.tile([128, RB], F32)
            nc.vector.tensor_tensor(out=lc[:], in0=lse[:], in1=p4[:], op=ALU.subtract)
            picked = small.tile([128, 1], F32)
            pscr = small.tile([128, RB, 4], F32)
            nc.vector.scalar_tensor_tensor(
                out=pscr[:], in0=rp_sb[:], scalar=1.0, in1=oh_sb[:],
                op0=ALU.mult, op1=ALU.mult, accum_out=picked[:],
            )
            csum = small.tile([128, 1], F32)
            nc.vector.reduce_sum(csum[:], lc[:], axis=mybir.AxisListType.X)
            rsum = small.tile([128, 1], F32)
            nc.vector.reduce_sum(rsum[:], rlse[:], axis=mybir.AxisListType.X)
            tot = small.tile([128, 1], F32)
            nc.vector.tensor_tensor(out=tot[:], in0=csum[:], in1=rsum[:], op=ALU.add)
            nc.vector.tensor_tensor(out=tot[:], in0=tot[:], in1=picked[:], op=ALU.subtract)

            psF = psp.tile([128, 2048], F32, tag="ps")
            nc.tensor.matmul(psF[0:1, 0:1], lhsT=tot[:], rhs=ones[:], start=True, stop=True)
            outsb = small.tile([1, 1], F32)
            nc.vector.tensor_copy(outsb[:], psF[0:1, 0:1])
            nc.sync.dma_start(out=partial[:], in_=outsb[:])

    nc.compile()
    return nc


def get_nc():
    if "nc" not in _CACHE:
        _CACHE["nc"] = _build()
    return _CACHE["nc"]


def _host_inputs(z, rotation_predictions, labels):
    import ml_dtypes

    z = np.ascontiguousarray(np.asarray(z, dtype=np.float32))
    rp = np.ascontiguousarray(np.asarray(rotation_predictions, dtype=np.float32))
    lab = np.asarray(labels).astype(np.int64)
    oh_full = np.eye(4, dtype=np.float32)[lab % 4]

    idm = np.eye(128, dtype=np.float32)
    idmb = np.eye(128, dtype=ml_dtypes.bfloat16)
    pidx = np.arange(128)
    pmk = np.zeros((128, 128), dtype=np.float32)
    pmk[pidx, pidx ^ 1] = 1.0

    in_maps = []
    for c in range(N_CORES):
        r0, r1 = c * SLAB, (c + 1) * SLAB
        in_maps.append(
            {
                "z": z,
                "zslab": z[r0:r1],
                "rp": rp[r0:r1],
                "oh": oh_full[r0:r1],
                "idm": idm,
                "idmb": idmb,
                "pm": pmk,
            }
        )
    return in_maps


def kernel(z, rotation_predictions, labels):
    nc = get_nc()
    in_maps = _host_inputs(z, rotation_predictions, labels)
    res = run_bass_kernel_spmd(nc, in_maps, core_ids=list(range(N_CORES)))
    total = sum(float(res.results[c]["partial"][0, 0]) for c in range(N_CORES))
    return np.float32(total / B)


if __name__ == "__main__":
    rng = np.random.default_rng(0)
    z = rng.standard_normal((B, D), dtype=np.float32)
    rp = rng.standard_normal((B, 4), dtype=np.float32)
    lab = rng.integers(0, 4, size=(B,)).astype(np.int64)
    print("loss:", kernel(z, rp, lab))


# revision 23
# speedup vs baseline: 1.5573x; 1.0096x over previous
"""CSILoss (contrastive + rotation CE) Trainium2 kernel.

Contract: kernel(**inputs) takes the FULL unsharded inputs
  z: [8192, 256] f32, rotation_predictions: [8192, 4] f32, labels: [8192] i64
and returns the full scalar loss (f32), computed on 8 NeuronCores.

Sharding: data-parallel over rows of z. Each core receives the full z (to
build the normalized-transposed embedding matrix znT used as the matmul RHS)
plus its own 1024-row slab (LHS source, rotation slab, label one-hots). Each
core computes its 1024x8192 cosine-similarity slab on the PE (fp8 DoubleRow),
exponentiates with fused row-sum accumulation on the scalar engine, extracts
the positive/diagonal terms from bitwise-identical recomputed diagonal
blocks, and reduces to one scalar partial; the host sums the 8 partials.

Engine split: GpSimd converts z to bf16; DVE computes row sums-of-squares
(fused mul+accum), rsqrt (Quake seed + Newton, no ACT tables), the rnorm
rescale, PSUM->fp8 copies and mask extractions; PE transposes via identity
matmuls and runs the fp8 logits matmuls; the scalar engine does Exp (and two
final Ln ops) with fused row-sum accumulation on the big exponentials.
Next-block transposes are interleaved between the exp slots so the scalar
engine stays saturated.
"""

import sys

for _p in ("/opt/trn_rl_repo", "/root/.axon_site/_ro/trn_rl_repo"):
    if _p not in sys.path:
        sys.path.insert(0, _p)

import numpy as np

import concourse.bass as bass
import concourse.tile as tile
from concourse import bacc, mybir
from concourse.bass import ds, ts
from concourse.bass_utils import run_bass_kernel_spmd

B, D = 8192, 256
N_CORES = 8
SLAB = B // N_CORES
RB = SLAB // 128
TB = B // 128
F32 = mybir.dt.float32
BF16 = mybir.dt.bfloat16
FP8 = mybir.dt.float8e4
AF = mybir.ActivationFunctionType
ALU = mybir.AluOpType
DR = mybir.MatmulPerfMode.DoubleRow

_CACHE = {}


def _build():
    nc = bacc.Bacc("TRN2", target_bir_lowering=False, debug=False)

    z = nc.declare_dram_parameter("z", [B, D], F32, isOutput=False)
    zslab = nc.declare_dram_parameter("zslab", [SLAB, D], F32, isOutput=False)
    rp = nc.declare_dram_parameter("rp", [SLAB, 4], F32, isOutput=False)
    oh = nc.declare_dram_parameter("oh", [SLAB, 4], F32, isOutput=False)
    idm = nc.declare_dram_parameter("idm", [128, 128], F32, isOutput=False)
    idmb = nc.declare_dram_parameter("idmb", [128, 128], BF16, isOutput=False)
    pm = nc.declare_dram_parameter("pm", [128, 128], F32, isOutput=False)
    partial = nc.declare_dram_parameter("partial", [1, 1], F32, isOutput=True)

    with tile.TileContext(nc) as tc:
        from contextlib import ExitStack

        with ExitStack() as stk:
            const = stk.enter_context(tc.tile_pool(name="const", bufs=1))
            small = stk.enter_context(tc.tile_pool(name="small", bufs=1))
            escp = stk.enter_context(tc.tile_pool(name="esc", bufs=2))
            zf32 = stk.enter_context(tc.tile_pool(name="zf32", bufs=4))
            zbfp = stk.enter_context(tc.tile_pool(name="zbfp", bufs=9))
            znp = stk.enter_context(tc.tile_pool(name="znp", bufs=9))
            sqp = stk.enter_context(tc.tile_pool(name="sqp", bufs=4))
            msc = stk.enter_context(tc.tile_pool(name="msc", bufs=2))
            psp = stk.enter_context(tc.tile_pool(name="psp", bufs=2, space="PSUM"))

            # ---- early z DMAs (before const DMAs: SP sequencer is serial)
            zs_f = zf32.tile([128, RB, D], F32, tag="zf")
            nc.sync.dma_start(
                out=zs_f[:], in_=zslab[:, :].rearrange("(b p) d -> p b d", p=128)
            )
            zfs = [None] * 8

            def dma_chunk(g):
                zf = zf32.tile([128, 8, D], F32, tag="zf", name=f"zf{g}")
                nc.sync.dma_start(
                    out=zf[:],
                    in_=z[g * 1024 : (g + 1) * 1024, :].rearrange(
                        "(b p) d -> p b d", p=128
                    ),
                )
                zfs[g] = zf

            dma_chunk(0)
            dma_chunk(1)

            # ---- constants / small inputs
            idm_sb = const.tile([128, 128], F32)
            nc.sync.dma_start(out=idm_sb[:], in_=idm[:])
            idmb_sb = const.tile([128, 128], BF16)
            nc.sync.dma_start(out=idmb_sb[:], in_=idmb[:])
            pm_sb = const.tile([128, 128], F32)
            nc.sync.dma_start(out=pm_sb[:], in_=pm[:])
            rp_sb = const.tile([128, RB, 4], F32)
            nc.sync.dma_start(out=rp_sb[:], in_=rp[:, :].rearrange("(b p) f -> p b f", p=128))
            oh_sb = const.tile([128, RB, 4], F32)
            nc.sync.dma_start(out=oh_sb[:], in_=oh[:, :].rearrange("(b p) f -> p b f", p=128))
            ones = const.tile([128, 1], F32)
            nc.vector.memset(ones[:], 1.0)

            znT8 = const.tile([128, 2, B], FP8, tag="znT8")
            zsT8 = const.tile([128, 2, SLAB], FP8, tag="zsT8")

            sumsq = small.tile([128, TB], F32)
            rnorm = small.tile([128, TB], F32)
            sumsq_s = small.tile([128, RB], F32)
            rnorm_s = small.tile([128, RB], F32)
            posv = small.tile([128, RB], F32)
            diagv = small.tile([128, RB], F32)
            acc = small.tile([128, RB, 4], F32)

            def sumsq_of(dst_col, src_ap):
                scr = sqp.tile([128, D], BF16, tag="sqscr")
                nc.vector.scalar_tensor_tensor(
                    out=scr[:], in0=src_ap, scalar=1.0, in1=src_ap,
                    op0=ALU.mult, op1=ALU.mult, accum_out=dst_col,
                )

            def rsqrt_of(dst_sl, src_sl, k):
                # dst = min(rsqrt(src), 1e8) entirely on DVE:
                # Quake-III seed + 2 Newton iterations (rel err ~5e-6).
                sb = src_sl.bitcast(mybir.dt.uint32)
                hbits = sqp.tile([128, k], mybir.dt.int32, tag=f"rsq_h{k}")
                nc.vector.tensor_scalar(
                    out=hbits[:].bitcast(mybir.dt.uint32), in0=sb, scalar1=1,
                    scalar2=None, op0=ALU.logical_shift_right,
                )
                seed = sqp.tile([128, k], mybir.dt.int32, tag=f"rsq_s{k}")
                nc.vector.tensor_scalar(
                    out=seed[:], in0=hbits[:], scalar1=-1, scalar2=0x5F3759DF,
                    op0=ALU.mult, op1=ALU.add,
                )
                y = seed[:].bitcast(F32)
                y2 = sqp.tile([128, k], F32, tag=f"rsq_y2{k}")
                w = sqp.tile([128, k], F32, tag=f"rsq_w{k}")
                for _ in range(2):
                    nc.vector.tensor_tensor(out=y2[:], in0=y, in1=y, op=ALU.mult)
                    nc.vector.scalar_tensor_tensor(
                        out=w[:], in0=y2[:], scalar=-0.5, in1=src_sl,
                        op0=ALU.mult, op1=ALU.mult,
                    )
                    nc.vector.tensor_scalar(
                        out=w[:], in0=w[:], scalar1=1.5, scalar2=None, op0=ALU.add
                    )
                    nc.vector.tensor_tensor(out=y, in0=y, in1=w[:], op=ALU.mult)
                nc.vector.tensor_scalar(
                    out=dst_sl, in0=y, scalar1=1e8, scalar2=None, op0=ALU.min
                )

            znbf = [None] * 8

            def proc_chunk(g, cvt="g"):
                zb = zbfp.tile([128, 8, D], BF16, tag="zbf", name=f"zbf{g}")
                (nc.gpsimd if cvt == "g" else nc.vector).tensor_copy(zb[:], zfs[g][:])
                for b in range(8):
                    t = 8 * g + b
                    sumsq_of(sumsq[:, t : t + 1], zb[:, b, :])
                znbf[g] = zb

            def scale_chunk(g):
                zn = znp.tile([128, 8, D], BF16, tag="znbf", name=f"znbf{g}")
                for b in range(8):
                    t = 8 * g + b
                    nc.vector.tensor_scalar_mul(
                        out=zn[:, b, :], in0=znbf[g][:, b, :], scalar1=rnorm[:, t : t + 1]
                    )
                znbf[g] = zn

            # transpose one chunk g (8 blocks, both d-halves) -> znT8 cols
            def emit_T(g):
                ps_t = psp.tile([128, 2048], F32, tag="ps", name=f"ps_t{g}")
                for b in range(8):
                    for h in range(2):
                        nc.tensor.matmul(
                            ps_t[:, ds(h * 1024 + b * 128, 128)],
                            lhsT=znbf[g][:, b, ds(h * 128, 128)],
                            rhs=idmb_sb[:],
                            start=True,
                            stop=True,
                        )
                for h in range(2):
                    nc.vector.tensor_copy(
                        znT8[:, h, ds(1024 * g, 1024)], ps_t[:, ds(h * 1024, 1024)]
                    )

            # ---- slab pipeline
            zs_b = zbfp.tile([128, RB, D], BF16, tag="zbf")
            nc.vector.tensor_copy(zs_b[:], zs_f[:])
            for b in range(RB):
                sumsq_of(sumsq_s[:, b : b + 1], zs_b[:, b, :])
            rsqrt_of(rnorm_s[:, :], sumsq_s[:, :], RB)
            zn_s = znp.tile([128, RB, D], BF16, tag="znbf")
            for b in range(RB):
                nc.vector.tensor_scalar_mul(
                    out=zn_s[:, b, :], in0=zs_b[:, b, :], scalar1=rnorm_s[:, b : b + 1]
                )
            ps_s = psp.tile([128, 2048], F32, tag="ps")
            for i in range(RB):
                for h in range(2):
                    nc.tensor.matmul(
                        ps_s[:, ds(h * 1024 + i * 128, 128)],
                        lhsT=zn_s[:, i, ds(h * 128, 128)],
                        rhs=idmb_sb[:],
                        start=True,
                        stop=True,
                    )
            for h in range(2):
                nc.vector.tensor_copy(zsT8[:, h, :], ps_s[:, ds(h * 1024, 1024)])

            # deferred diagonal-block extraction (runs in chunk 3's spare slot)
            ed = small.tile([128, RB], F32)
            dcp = const.tile([128, RB, 128], F32)

            def emit_diag_blocks():
                ps_d = psp.tile([128, 2048], F32, tag="ps")
                for rb in range(RB):
                    nc.tensor.matmul(
                        ps_d[:, ts(rb, 128)],
                        lhsT=zsT8[:, :, ts(rb, 128)],
                        rhs=zsT8[:, :, ts(rb, 128)],
                        start=True,
                        stop=True,
                        perf_mode=DR,
                    )
                nc.vector.tensor_copy(
                    dcp[:], ps_d[:, 0:1024].rearrange("p (i c) -> p i c", c=128)
                )
                for rb in range(RB):
                    mscr = msc.tile([128, 128], F32, tag="mscr")
                    nc.vector.scalar_tensor_tensor(
                        out=mscr[:], in0=dcp[:, rb, :], scalar=1.0, in1=pm_sb[:],
                        op0=ALU.mult, op1=ALU.mult, accum_out=posv[:, rb : rb + 1],
                    )
                    mscr2 = msc.tile([128, 128], F32, tag="mscr")
                    nc.vector.scalar_tensor_tensor(
                        out=mscr2[:], in0=dcp[:, rb, :], scalar=1.0, in1=idm_sb[:],
                        op0=ALU.mult, op1=ALU.mult, accum_out=diagv[:, rb : rb + 1],
                    )
                nc.scalar.activation(out=ed[:], in_=diagv[:], func=AF.Exp, scale=4.0)

            # ---- chunks 0/1 -> first transposes
            proc_chunk(0)
            rsqrt_of(rnorm[:, 0:8], sumsq[:, 0:8], 8)
            scale_chunk(0)
            emit_T(0)
            proc_chunk(1)
            rsqrt_of(rnorm[:, 8:16], sumsq[:, 8:16], 8)
            scale_chunk(1)
            emit_T(1)

            # rotation exps early (ACT otherwise idle in the lead-in)
            rs = small.tile([128, RB], F32)
            rescr = small.tile([128, RB, 4], F32)
            for b in range(RB):
                nc.scalar.activation(
                    out=rescr[:, b, :],
                    in_=rp_sb[:, b, :],
                    func=AF.Exp,
                    accum_out=rs[:, b : b + 1],
                )

            dma_chunk(2)
            proc_chunk(2)
            dma_chunk(3)
            proc_chunk(3)
            rsqrt_of(rnorm[:, 16:32], sumsq[:, 16:32], 16)

            # ---- streamed chunks: big matmuls + exp; next transposes between slots
            for n in range(4):
                for rb in range(RB):
                    ps = psp.tile([128, 2048], F32, tag="ps")
                    for s in range(4):
                        nc.tensor.matmul(
                            ps[:, ts(s, 512)],
                            lhsT=zsT8[:, :, ts(rb, 128)],
                            rhs=znT8[:, :, ds(2048 * n + 512 * s, 512)],
                            start=True,
                            stop=True,
                            perf_mode=DR,
                        )
                    e = escp.tile([128, 2048], BF16, tag="esc")
                    nc.scalar.activation(
                        out=e[:],
                        in_=ps[:],
                        func=AF.Exp,
                        scale=4.0,
                        accum_out=acc[:, rb, n : n + 1],
                    )
                    if n < 3 and rb == 3:
                        scale_chunk(2 * n + 2)
                    if n < 3 and rb == 4:
                        emit_T(2 * n + 2)
                    if n < 3 and rb == 5:
                        scale_chunk(2 * n + 3)
                    if n < 3 and rb == 6:
                        emit_T(2 * n + 3)
                    if n == 3 and rb == 4:
                        emit_diag_blocks()
                if n == 0:
                    for g in range(4, 8):
                        dma_chunk(g)
                        proc_chunk(g)
                    rsqrt_of(rnorm[:, 32:48], sumsq[:, 32:48], 16)
                    rsqrt_of(rnorm[:, 48:64], sumsq[:, 48:64], 16)

            # ---- finals (Ln ops grouped at the very end)
            S = small.tile([128, RB], F32)
            nc.vector.reduce_sum(S[:], acc[:], axis=mybir.AxisListType.X)
            Sm = small.tile([128, RB], F32)
            nc.vector.tensor_tensor(out=Sm[:], in0=S[:], in1=ed[:], op=ALU.subtract)
            lse = small.tile([128, RB], F32)
            nc.scalar.activation(out=lse[:], in_=Sm[:], func=AF.Ln)
            rlse = small.tile([128, RB], F32)
            nc.scalar.activation(out=rlse[:], in_=rs[:], func=AF.Ln)

            p4 = small.tile([128, RB], F32)
            nc.vector.tensor_scalar_mul(out=p4[:], in0=posv[:], scalar1=4.0)
            lc = small.tile([128, RB], F32)
            nc.vector.tensor_tensor(out=lc[:], in0=lse[:], in1=p4[:], op=ALU.subtract)
            picked = small.tile([128, 1], F32)
            pscr = small.tile([128, RB, 4], F32)
            nc.vector.scalar_tensor_tensor(
                out=pscr[:], in0=rp_sb[:], scalar=1.0, in1=oh_sb[:],
                op0=ALU.mult, op1=ALU.mult, accum_out=picked[:],
            )
            csum = small.tile([128, 1], F32)
            nc.vector.reduce_sum(csum[:], lc[:], axis=mybir.AxisListType.X)
            rsum = small.tile([128, 1], F32)
            nc.vector.reduce_sum(rsum[:], rlse[:], axis=mybir.AxisListType.X)
            tot = small.tile([128, 1], F32)
            nc.vector.tensor_tensor(out=tot[:], in0=csum[:], in1=rsum[:], op=ALU.add)
            nc.vector.tensor_tensor(out=tot[:], in0=tot[:], in1=picked[:], op=ALU.subtract)

            psF = psp.tile([128, 2048], F32, tag="ps")
            nc.tensor.matmul(psF[0:1, 0:1], lhsT=tot[:], rhs=ones[:], start=True, stop=True)
            outsb = small.tile([1, 1], F32)
            nc.vector.tensor_copy(outsb[:], psF[0:1, 0:1])
            nc.sync.dma_start(out=partial[:], in_=outsb[:])

    nc.compile()
    return nc


def get_nc():
    if "nc" not in _CACHE:
        _CACHE["nc"] = _build()
    return _CACHE["nc"]


def _host_inputs(z, rotation_predictions, labels):
    import ml_dtypes

    z = np.ascontiguousarray(np.asarray(z, dtype=np.float32))
    rp = np.ascontiguousarray(np.asarray(rotation_predictions, dtype=np.float32))
    lab = np.asarray(labels).astype(np.int64)
    oh_full = np.eye(4, dtype=np.float32)[lab % 4]

    idm = np.eye(128, dtype=np.float32)
    idmb = np.eye(128, dtype=ml_dtypes.bfloat16)
    pidx = np.arange(128)
    pmk = np.zeros((128, 128), dtype=np.float32)
    pmk[pidx, pidx ^ 1] = 1.0

    in_maps = []
    for c in range(N_CORES):
        r0, r1 = c * SLAB, (c + 1) * SLAB
        in_maps.append(
            {
                "z": z,
                "zslab": z[r0:r1],
                "rp": rp[r0:r1],
                "oh": oh_full[r0:r1],
                "idm": idm,
                "idmb": idmb,
                "pm": pmk,
            }
        )
    return in_maps


def kernel(z, rotation_predictions, labels):
    nc = get_nc()
    in_maps = _host_inputs(z, rotation_predictions, labels)
    res = run_bass_kernel_spmd(nc, in_maps, core_ids=list(range(N_CORES)))
    total = sum(float(res.results[c]["partial"][0, 0]) for c in range(N_CORES))
    return np.float32(total / B)


if __name__ == "__main__":
    rng = np.random.default_rng(0)
    z = rng.standard_normal((B, D), dtype=np.float32)
    rp = rng.standard_normal((B, 4), dtype=np.float32)
    lab = rng.integers(0, 4, size=(B,)).astype(np.int64)
    print("loss:", kernel(z, rp, lab))


# revision 24
# speedup vs baseline: 1.6535x; 1.0618x over previous
"""CSILoss (contrastive + rotation CE) Trainium2 kernel.

Contract: kernel(**inputs) takes the FULL unsharded inputs
  z: [8192, 256] f32, rotation_predictions: [8192, 4] f32, labels: [8192] i64
and returns the full scalar loss (f32), computed on 8 NeuronCores.

Sharding: data-parallel over rows of z. Each core receives the full z (to
build the normalized-transposed embedding matrix znT used as the matmul RHS)
plus its own 1024-row slab (LHS source, rotation slab, label one-hots). Each
core computes its 1024x8192 cosine-similarity slab on the PE (fp8 DoubleRow),
exponentiates with fused row-sum accumulation on the scalar engine, extracts
the positive/diagonal terms from bitwise-identical recomputed diagonal
blocks, and reduces to one scalar partial; the host sums the 8 partials.

Engine split: GpSimd converts z to bf16; DVE computes row sums-of-squares
(fused mul+accum), rsqrt (Quake seed + Newton, no ACT table switches), the
diag(rnorm) tiles, steady-state PSUM->fp8 copies and mask extractions; PE
transposes via z_blockT @ diag(rnorm) matmuls and runs the fp8 logits
matmuls; the scalar engine runs Exp (+ two final Ln) with fused row-sum
accumulation, plus the lead-in PSUM->fp8 copies while it is otherwise idle.
Transposes for upcoming chunks are interleaved between exp slots.
"""

import sys

for _p in ("/opt/trn_rl_repo", "/root/.axon_site/_ro/trn_rl_repo"):
    if _p not in sys.path:
        sys.path.insert(0, _p)

import numpy as np

import concourse.bass as bass
import concourse.tile as tile
from concourse import bacc, mybir
from concourse.bass import ds, ts
from concourse.bass_utils import run_bass_kernel_spmd

B, D = 8192, 256
N_CORES = 8
SLAB = B // N_CORES
RB = SLAB // 128
TB = B // 128
F32 = mybir.dt.float32
BF16 = mybir.dt.bfloat16
FP8 = mybir.dt.float8e4
AF = mybir.ActivationFunctionType
ALU = mybir.AluOpType
DR = mybir.MatmulPerfMode.DoubleRow

_CACHE = {}


def _build():
    nc = bacc.Bacc("TRN2", target_bir_lowering=False, debug=False)

    z = nc.declare_dram_parameter("z", [B, D], F32, isOutput=False)
    zslab = nc.declare_dram_parameter("zslab", [SLAB, D], F32, isOutput=False)
    rp = nc.declare_dram_parameter("rp", [SLAB, 4], F32, isOutput=False)
    oh = nc.declare_dram_parameter("oh", [SLAB, 4], F32, isOutput=False)
    idm = nc.declare_dram_parameter("idm", [128, 128], F32, isOutput=False)
    idmb = nc.declare_dram_parameter("idmb", [128, 128], BF16, isOutput=False)
    pm = nc.declare_dram_parameter("pm", [128, 128], F32, isOutput=False)
    partial = nc.declare_dram_parameter("partial", [1, 1], F32, isOutput=True)

    with tile.TileContext(nc) as tc:
        from contextlib import ExitStack

        with ExitStack() as stk:
            const = stk.enter_context(tc.tile_pool(name="const", bufs=1))
            small = stk.enter_context(tc.tile_pool(name="small", bufs=1))
            escp = stk.enter_context(tc.tile_pool(name="esc", bufs=2))
            zf32 = stk.enter_context(tc.tile_pool(name="zf32", bufs=4))
            zbfp = stk.enter_context(tc.tile_pool(name="zbfp", bufs=9))
            drp = stk.enter_context(tc.tile_pool(name="drp", bufs=12))
            sqp = stk.enter_context(tc.tile_pool(name="sqp", bufs=4))
            msc = stk.enter_context(tc.tile_pool(name="msc", bufs=2))
            psp = stk.enter_context(tc.tile_pool(name="psp", bufs=2, space="PSUM"))

            # ---- early z DMAs (before const DMAs: SP sequencer is serial)
            zs_f = zf32.tile([128, RB, D], F32, tag="zf")
            nc.sync.dma_start(
                out=zs_f[:], in_=zslab[:, :].rearrange("(b p) d -> p b d", p=128)
            )
            zfs = [None] * 8

            def dma_chunk(g):
                zf = zf32.tile([128, 8, D], F32, tag="zf", name=f"zf{g}")
                nc.sync.dma_start(
                    out=zf[:],
                    in_=z[g * 1024 : (g + 1) * 1024, :].rearrange(
                        "(b p) d -> p b d", p=128
                    ),
                )
                zfs[g] = zf

            dma_chunk(0)
            dma_chunk(1)

            # ---- constants / small inputs
            idm_sb = const.tile([128, 128], F32)
            nc.sync.dma_start(out=idm_sb[:], in_=idm[:])
            idmb_sb = const.tile([128, 128], BF16)
            nc.sync.dma_start(out=idmb_sb[:], in_=idmb[:])
            pm_sb = const.tile([128, 128], F32)
            nc.sync.dma_start(out=pm_sb[:], in_=pm[:])
            rp_sb = const.tile([128, RB, 4], F32)
            nc.sync.dma_start(out=rp_sb[:], in_=rp[:, :].rearrange("(b p) f -> p b f", p=128))
            oh_sb = const.tile([128, RB, 4], F32)
            nc.sync.dma_start(out=oh_sb[:], in_=oh[:, :].rearrange("(b p) f -> p b f", p=128))
            ones = const.tile([128, 1], F32)
            nc.vector.memset(ones[:], 1.0)

            znT8 = const.tile([128, 2, B], FP8, tag="znT8")
            zsT8 = const.tile([128, 2, SLAB], FP8, tag="zsT8")

            sumsq = small.tile([128, TB], F32)
            rnorm = small.tile([128, TB], F32)
            sumsq_s = small.tile([128, RB], F32)
            rnorm_s = small.tile([128, RB], F32)
            posv = small.tile([128, RB], F32)
            diagv = small.tile([128, RB], F32)
            acc = small.tile([128, RB, 4], F32)

            def sumsq_of(dst_col, src_ap):
                scr = sqp.tile([128, D], BF16, tag="sqscr")
                nc.vector.scalar_tensor_tensor(
                    out=scr[:], in0=src_ap, scalar=1.0, in1=src_ap,
                    op0=ALU.mult, op1=ALU.mult, accum_out=dst_col,
                )

            def rsqrt_of(dst_sl, src_sl, k):
                # dst = min(rsqrt(src), 1e8) entirely on DVE:
                # Quake-III seed + 2 Newton iterations (rel err ~5e-6).
                sb = src_sl.bitcast(mybir.dt.uint32)
                hbits = sqp.tile([128, k], mybir.dt.int32, tag=f"rsq_h{k}")
                nc.vector.tensor_scalar(
                    out=hbits[:].bitcast(mybir.dt.uint32), in0=sb, scalar1=1,
                    scalar2=None, op0=ALU.logical_shift_right,
                )
                seed = sqp.tile([128, k], mybir.dt.int32, tag=f"rsq_s{k}")
                nc.vector.tensor_scalar(
                    out=seed[:], in0=hbits[:], scalar1=-1, scalar2=0x5F3759DF,
                    op0=ALU.mult, op1=ALU.add,
                )
                y = seed[:].bitcast(F32)
                y2 = sqp.tile([128, k], F32, tag=f"rsq_y2{k}")
                w = sqp.tile([128, k], F32, tag=f"rsq_w{k}")
                for _ in range(2):
                    nc.vector.tensor_tensor(out=y2[:], in0=y, in1=y, op=ALU.mult)
                    nc.vector.scalar_tensor_tensor(
                        out=w[:], in0=y2[:], scalar=-0.5, in1=src_sl,
                        op0=ALU.mult, op1=ALU.mult,
                    )
                    nc.vector.tensor_scalar(
                        out=w[:], in0=w[:], scalar1=1.5, scalar2=None, op0=ALU.add
                    )
                    nc.vector.tensor_tensor(out=y, in0=y, in1=w[:], op=ALU.mult)
                nc.vector.tensor_scalar(
                    out=dst_sl, in0=y, scalar1=1e8, scalar2=None, op0=ALU.min
                )

            zbf = [None] * 8

            def proc_chunk(g):
                zb = zbfp.tile([128, 8, D], BF16, tag="zbf", name=f"zbf{g}")
                nc.gpsimd.tensor_copy(zb[:], zfs[g][:])
                for b in range(8):
                    t = 8 * g + b
                    sumsq_of(sumsq[:, t : t + 1], zb[:, b, :])
                zbf[g] = zb

            # transpose+normalize chunk g: znT[:, t*128+j] = zbf[row j of t]*rnorm_j
            def emit_T(g, copy_eng="v"):
                drs = []
                for b in range(8):
                    t = 8 * g + b
                    dr_t = drp.tile([128, 128], BF16, tag="dr", name=f"dr{g}_{b}")
                    nc.vector.tensor_scalar_mul(
                        out=dr_t[:], in0=idmb_sb[:], scalar1=rnorm[:, t : t + 1]
                    )
                    drs.append(dr_t)
                ps_t = psp.tile([128, 2048], F32, tag="ps", name=f"ps_t{g}")
                for b in range(8):
                    for h in range(2):
                        nc.tensor.matmul(
                            ps_t[:, ds(h * 1024 + b * 128, 128)],
                            lhsT=zbf[g][:, b, ds(h * 128, 128)],
                            rhs=drs[b][:],
                            start=True,
                            stop=True,
                        )
                eng = nc.vector if copy_eng == "v" else nc.scalar
                for h in range(2):
                    if copy_eng == "v":
                        nc.vector.tensor_copy(
                            znT8[:, h, ds(1024 * g, 1024)], ps_t[:, ds(h * 1024, 1024)]
                        )
                    else:
                        nc.scalar.copy(
                            znT8[:, h, ds(1024 * g, 1024)], ps_t[:, ds(h * 1024, 1024)]
                        )

            # ---- slab pipeline
            zs_b = zbfp.tile([128, RB, D], BF16, tag="zbf")
            nc.gpsimd.tensor_copy(zs_b[:], zs_f[:])
            for b in range(RB):
                sumsq_of(sumsq_s[:, b : b + 1], zs_b[:, b, :])
            rsqrt_of(rnorm_s[:, :], sumsq_s[:, :], RB)
            ps_s = psp.tile([128, 2048], F32, tag="ps")
            for i in range(RB):
                dr_s = drp.tile([128, 128], BF16, tag="dr", name=f"drs{i}")
                nc.vector.tensor_scalar_mul(
                    out=dr_s[:], in0=idmb_sb[:], scalar1=rnorm_s[:, i : i + 1]
                )
                for h in range(2):
                    nc.tensor.matmul(
                        ps_s[:, ds(h * 1024 + i * 128, 128)],
                        lhsT=zs_b[:, i, ds(h * 128, 128)],
                        rhs=dr_s[:],
                        start=True,
                        stop=True,
                    )
            for h in range(2):
                nc.scalar.copy(zsT8[:, h, :], ps_s[:, ds(h * 1024, 1024)])

            # deferred diagonal-block extraction (runs in chunk 3's spare slot)
            ed = small.tile([128, RB], F32)
            dcp = const.tile([128, RB, 128], F32)

            def emit_diag_blocks():
                ps_d = psp.tile([128, 2048], F32, tag="ps")
                for rb in range(RB):
                    nc.tensor.matmul(
                        ps_d[:, ts(rb, 128)],
                        lhsT=zsT8[:, :, ts(rb, 128)],
                        rhs=zsT8[:, :, ts(rb, 128)],
                        start=True,
                        stop=True,
                        perf_mode=DR,
                    )
                nc.vector.tensor_copy(
                    dcp[:], ps_d[:, 0:1024].rearrange("p (i c) -> p i c", c=128)
                )
                for rb in range(RB):
                    mscr = msc.tile([128, 128], F32, tag="mscr")
                    nc.vector.scalar_tensor_tensor(
                        out=mscr[:], in0=dcp[:, rb, :], scalar=1.0, in1=pm_sb[:],
                        op0=ALU.mult, op1=ALU.mult, accum_out=posv[:, rb : rb + 1],
                    )
                    mscr2 = msc.tile([128, 128], F32, tag="mscr")
                    nc.vector.scalar_tensor_tensor(
                        out=mscr2[:], in0=dcp[:, rb, :], scalar=1.0, in1=idm_sb[:],
                        op0=ALU.mult, op1=ALU.mult, accum_out=diagv[:, rb : rb + 1],
                    )
                nc.scalar.activation(out=ed[:], in_=diagv[:], func=AF.Exp, scale=4.0)

            # ---- chunks 0/1 -> first transposes (copies on ACT: idle in lead-in)
            proc_chunk(0)
            rsqrt_of(rnorm[:, 0:8], sumsq[:, 0:8], 8)
            emit_T(0, copy_eng="s")
            proc_chunk(1)
            rsqrt_of(rnorm[:, 8:16], sumsq[:, 8:16], 8)
            emit_T(1, copy_eng="s")

            # rotation exps early (ACT otherwise idle in the lead-in)
            rs = small.tile([128, RB], F32)
            rescr = small.tile([128, RB, 4], F32)
            for b in range(RB):
                nc.scalar.activation(
                    out=rescr[:, b, :],
                    in_=rp_sb[:, b, :],
                    func=AF.Exp,
                    accum_out=rs[:, b : b + 1],
                )

            dma_chunk(2)
            proc_chunk(2)
            dma_chunk(3)
            proc_chunk(3)
            rsqrt_of(rnorm[:, 16:32], sumsq[:, 16:32], 16)

            # ---- streamed chunks: big matmuls + exp; next transposes between slots
            for n in range(4):
                for rb in range(RB):
                    ps = psp.tile([128, 2048], F32, tag="ps")
                    for s in range(4):
                        nc.tensor.matmul(
                            ps[:, ts(s, 512)],
                            lhsT=zsT8[:, :, ts(rb, 128)],
                            rhs=znT8[:, :, ds(2048 * n + 512 * s, 512)],
                            start=True,
                            stop=True,
                            perf_mode=DR,
                        )
                    e = escp.tile([128, 2048], BF16, tag="esc")
                    nc.scalar.activation(
                        out=e[:],
                        in_=ps[:],
                        func=AF.Exp,
                        scale=4.0,
                        accum_out=acc[:, rb, n : n + 1],
                    )
                    if n < 3 and rb == 4:
                        emit_T(2 * n + 2)
                    if n < 3 and rb == 6:
                        emit_T(2 * n + 3)
                    if n == 3 and rb == 4:
                        emit_diag_blocks()
                if n == 0:
                    for g in range(4, 8):
                        dma_chunk(g)
                        proc_chunk(g)
                    rsqrt_of(rnorm[:, 32:48], sumsq[:, 32:48], 16)
                    rsqrt_of(rnorm[:, 48:64], sumsq[:, 48:64], 16)

            # ---- finals (Ln ops grouped at the very end)
            S = small.tile([128, RB], F32)
            nc.vector.reduce_sum(S[:], acc[:], axis=mybir.AxisListType.X)
            Sm = small.tile([128, RB], F32)
            nc.vector.tensor_tensor(out=Sm[:], in0=S[:], in1=ed[:], op=ALU.subtract)
            lse = small.tile([128, RB], F32)
            nc.scalar.activation(out=lse[:], in_=Sm[:], func=AF.Ln)
            rlse = small.tile([128, RB], F32)
            nc.scalar.activation(out=rlse[:], in_=rs[:], func=AF.Ln)

            p4 = small.tile([128, RB], F32)
            nc.vector.tensor_scalar_mul(out=p4[:], in0=posv[:], scalar1=4.0)
            lc = small.tile([128, RB], F32)
            nc.vector.tensor_tensor(out=lc[:], in0=lse[:], in1=p4[:], op=ALU.subtract)
            picked = small.tile([128, 1], F32)
            pscr = small.tile([128, RB, 4], F32)
            nc.vector.scalar_tensor_tensor(
                out=pscr[:], in0=rp_sb[:], scalar=1.0, in1=oh_sb[:],
                op0=ALU.mult, op1=ALU.mult, accum_out=picked[:],
            )
            csum = small.tile([128, 1], F32)
            nc.vector.reduce_sum(csum[:], lc[:], axis=mybir.AxisListType.X)
            rsum = small.tile([128, 1], F32)
            nc.vector.reduce_sum(rsum[:], rlse[:], axis=mybir.AxisListType.X)
            tot = small.tile([128, 1], F32)
            nc.vector.tensor_tensor(out=tot[:], in0=csum[:], in1=rsum[:], op=ALU.add)
            nc.vector.tensor_tensor(out=tot[:], in0=tot[:], in1=picked[:], op=ALU.subtract)

            psF = psp.tile([128, 2048], F32, tag="ps")
            nc.tensor.matmul(psF[0:1, 0:1], lhsT=tot[:], rhs=ones[:], start=True, stop=True)
            outsb = small.tile([1, 1], F32)
            nc.vector.tensor_copy(outsb[:], psF[0:1, 0:1])
            nc.sync.dma_start(out=partial[:], in_=outsb[:])

    nc.compile()
    return nc


def get_nc():
    if "nc" not in _CACHE:
        _CACHE["nc"] = _build()
    return _CACHE["nc"]


def _host_inputs(z, rotation_predictions, labels):
    import ml_dtypes

    z = np.ascontiguousarray(np.asarray(z, dtype=np.float32))
    rp = np.ascontiguousarray(np.asarray(rotation_predictions, dtype=np.float32))
    lab = np.asarray(labels).astype(np.int64)
    oh_full = np.eye(4, dtype=np.float32)[lab % 4]

    idm = np.eye(128, dtype=np.float32)
    idmb = np.eye(128, dtype=ml_dtypes.bfloat16)
    pidx = np.arange(128)
    pmk = np.zeros((128, 128), dtype=np.float32)
    pmk[pidx, pidx ^ 1] = 1.0

    in_maps = []
    for c in range(N_CORES):
        r0, r1 = c * SLAB, (c + 1) * SLAB
        in_maps.append(
            {
                "z": z,
                "zslab": z[r0:r1],
                "rp": rp[r0:r1],
                "oh": oh_full[r0:r1],
                "idm": idm,
                "idmb": idmb,
                "pm": pmk,
            }
        )
    return in_maps


def kernel(z, rotation_predictions, labels):
    nc = get_nc()
    in_maps = _host_inputs(z, rotation_predictions, labels)
    res = run_bass_kernel_spmd(nc, in_maps, core_ids=list(range(N_CORES)))
    total = sum(float(res.results[c]["partial"][0, 0]) for c in range(N_CORES))
    return np.float32(total / B)


if __name__ == "__main__":
    rng = np.random.default_rng(0)
    z = rng.standard_normal((B, D), dtype=np.float32)
    rp = rng.standard_normal((B, 4), dtype=np.float32)
    lab = rng.integers(0, 4, size=(B,)).astype(np.int64)
    print("loss:", kernel(z, rp, lab))
